# revision 44
# baseline (speedup 1.0000x reference)
"""AgentSelfAttention1d Trainium2 kernel (v3).

Per batch b (one NeuronCore each):
    xt = x[b].T                       # [L=4096, D=512]
    q/k/v = xt @ W{q,k,v}.T + b       # [L, D]
    a  = AdaptiveAvgPool(q) -> [P=128, D]
    c  = softmax(a @ k.T, -1) @ v     # [P, D]
    r  = softmax(q @ a.T, -1) @ c     # [L, D]
    out[b] = r.T                      # [D, L]

Restructuring (all projections folded into host-precomputed weight
products; everything channel-first on chip):
    xp[c,p]   = seg-sum of x over 32-wide windows      (tiny PE matmuls
                against a one-hot segment indicator, from the x.T tiles)
    H[e,p]    = MKs[c,e]^T-contract xp + vk[e],  MKs = (Wq^T Wk)/32,
                vk = Wk^T bq          (S1[p,l] = sum_e H[e,p] x[e,l])
    G[e,p]    = MQs-contract xp + vq[e],         MQs = (Wq^T Wq)/32
    hq[p]     = (xp^T (Wq^T bq))/32 + |bq|^2     (S2[p,l] = G-part + hq)
    E1        = exp(S1 - 10)  bf16; rowsum via activation accumulator
    E2        = exp(S2 - 40)  bf16 (unnormalized; bf16 keeps the f32
                exponent range so all-underflow columns cannot occur)
    M1T[e,p]  = (E1 @ x.T)^T, accumulated directly transposed across four
                PSUM banks; E1 transposed by the DMA xbar engine
    cbv[p,d]  = (M1 @ Wv^T) / rowsum1 + bv
    out[l,d]  = (sum_p E2[p,l] cbv[p,d]) / colsum2[l]
    Output written [L, D] fp16; host transposes/upcasts to [D, L] f32.

v3 schedule: x DMAs are issued first (ident ahead of them on the sync
queue) so transposes start ~2.5us in; all 8 chunks are PE-transposed and
pooled uniformly during the x-load window (no DVE/xbar special case for
chunk 7, which previously sat on the critical path after the last x
chunk); pass 1 interleaves S1 and S2 per chunk (E1 and E2 exps spread
across the whole pass on the Act engine) with M1 trailing two chunks;
pass 2 is only the 32 output matmuls with colsum2 (rs2) one chunk ahead,
scales round-robined over Act/DVE/GpSimd, and the final chunk stored as
four small DMAs to cut the tail.
"""

import numpy as np
import ml_dtypes

import concourse.bass as bass
import concourse.mybir as mybir
import concourse.tile as tile
from concourse import bacc
from concourse.bass_utils import run_bass_kernel_spmd

F32 = mybir.dt.float32
F16 = mybir.dt.float16
BF16 = mybir.dt.bfloat16

B, D, L, P = 8, 512, 4096, 128
KT = D // 128      # 4 contraction tiles of 128
NCH = L // 512     # 8 l-chunks of 512
NLT = L // 128     # 32 l-tiles of 128
SEG = L // P       # 32: pool segment length
SHIFT1 = 10.0
SHIFT2 = 40.0

_CACHE = {}


def build():
    nc = bacc.Bacc(target_bir_lowering=False, trn_type="TRN2")
    X = nc.dram_tensor("x", [D, L], F16, kind="ExternalInput")
    MKS = nc.dram_tensor("mks", [D, D], F16, kind="ExternalInput")   # (Wq^T Wk)/32 [c,e]
    MQS = nc.dram_tensor("mqs", [D, D], F16, kind="ExternalInput")   # (Wq^T Wq)/32 [c,e]
    WVT = nc.dram_tensor("wvt", [D, D], BF16, kind="ExternalInput")  # Wv^T [e,d]
    IDN = nc.dram_tensor("ident", [128, 128], F16, kind="ExternalInput")
    INDS = nc.dram_tensor("inds", [128, 4], BF16, kind="ExternalInput")  # l -> l//32 one-hot
    ONES2 = nc.dram_tensor("ones2", [128, 2], BF16, kind="ExternalInput")
    VKR = nc.dram_tensor("vkr", [D], F16, kind="ExternalInput")      # Wk^T bq row
    VQR = nc.dram_tensor("vqr", [D], F16, kind="ExternalInput")      # Wq^T bq row
    VQ2 = nc.dram_tensor("vq2", [D, 2], F16, kind="ExternalInput")   # [(Wq^T bq)/32, 0]
    HB2 = nc.dram_tensor("hb2", [2], F16, kind="ExternalInput")      # [|bq|^2-S2, 0]
    ONESR = nc.dram_tensor("onesr", [128], F16, kind="ExternalInput")
    BVB = nc.dram_tensor("bvb", [128, D], BF16, kind="ExternalInput")     # bv bcast
    OUT = nc.dram_tensor("out", [L, D], F16, kind="ExternalOutput")

    from contextlib import ExitStack
    with nc.allow_low_precision("16-bit matmul operands"), \
         tile.TileContext(nc, pool_alloc_mode="queue") as tc, ExitStack() as stack:
        sb = stack.enter_context(tc.tile_pool(name="sb", bufs=1))
        e1p = stack.enter_context(tc.tile_pool(name="e1p", bufs=8))
        e1tp = stack.enter_context(tc.tile_pool(name="e1tp", bufs=8))
        e2p = stack.enter_context(tc.tile_pool(name="e2p", bufs=8))
        outp = stack.enter_context(tc.tile_pool(name="outp", bufs=8))
        iv2p = stack.enter_context(tc.tile_pool(name="iv2p", bufs=8))
        # PSUM (8 banks): load: xpps 1 + tp ring | prep: h/g/hq ring |
        # pass1: s1+s2 ring 4 (psA) + m1t 4 (psB) | pass2: out 4 + rs2
        psA = stack.enter_context(tc.tile_pool(name="psA", bufs=4, space="PSUM"))
        psB = stack.enter_context(tc.tile_pool(name="psB", bufs=4, space="PSUM"))

        # ---- ACT table warmup ------------------------------------------------
        warm = sb.tile([128, 1], F32)
        nc.vector.memset(warm, 0.0)
        nc.scalar.activation(out=warm, in_=warm,
                             func=mybir.ActivationFunctionType.Exp,
                             bias=warm, scale=1.0)

        # ---- input DMAs: x first on sync; ident leads the SWDGE queue --------
        ident = sb.tile([128, 128], F16)
        nc.gpsimd.dma_start(out=ident, in_=IDN[:, :])
        x_sb = sb.tile([128, KT, L], F16)
        xr = X.rearrange("(k p) l -> p k l", p=128)
        # halves 12-15 (chunks 6,7) first: those chunks pool via DVE reduces,
        # which serialize on DVE — early arrival hides that; the pool_mm
        # chunks (0-5) arrive last but have their chain split across engines.
        # mks arrives after x with a zero-length post-arrival chain (H fires
        # straight off its sem).
        H_ORDER = [12, 13, 14, 15] + list(range(12))
        for hch in H_ORDER:
            nc.sync.dma_start(out=x_sb[:, :, bass.ts(hch, 256)],
                              in_=xr[:, :, bass.ts(hch, 256)])
        # small tensors ride the SWDGE queue (prep overlaps the x stream)
        inds = sb.tile([128, 4], BF16)
        nc.gpsimd.dma_start(out=inds, in_=INDS[:, :])
        vkr = sb.tile([1, D], F16)
        nc.gpsimd.dma_start(out=vkr, in_=VKR.rearrange("(o d) -> o d", o=1))
        vqr = sb.tile([1, D], F16)
        nc.gpsimd.dma_start(out=vqr, in_=VQR.rearrange("(o d) -> o d", o=1))
        vq2 = sb.tile([128, KT, 2], F16)
        nc.gpsimd.dma_start(out=vq2, in_=VQ2.rearrange("(k p) t -> p k t", p=128))
        hb2 = sb.tile([1, 2], F16)
        nc.gpsimd.dma_start(out=hb2, in_=HB2.rearrange("(o d) -> o d", o=1))
        onesr = sb.tile([1, 128], F16)
        nc.gpsimd.dma_start(out=onesr, in_=ONESR.rearrange("(o d) -> o d", o=1))
        # weights after x on the sync queue
        mks = sb.tile([128, KT, D], F16)
        nc.sync.dma_start(out=mks, in_=MKS.rearrange("(k p) e -> p k e", p=128))
        mqs = sb.tile([128, KT, D], F16)
        nc.sync.dma_start(out=mqs, in_=MQS.rearrange("(k p) e -> p k e", p=128))
        wvt = sb.tile([128, KT, D], BF16)
        nc.sync.dma_start(out=wvt, in_=WVT.rearrange("(k p) e -> p k e", p=128))
        ones2 = sb.tile([128, 2], BF16)
        nc.sync.dma_start(out=ones2, in_=ONES2[:, :])
        bvb = sb.tile([128, D], BF16)
        nc.sync.dma_start(out=bvb, in_=BVB[:, :])
        sh1 = sb.tile([128, 1], F32)
        nc.vector.memset(sh1, -SHIFT1)

        # ---- x.T tiles (PE transpose) + pooling (tiny PE matmuls) ------------
        # All 8 chunks uniformly; runs inside the x-load window where PE,
        # Act, DVE and GpSimd are otherwise idle.
        xt = sb.tile([128, NLT, D], BF16)
        xpps = psA.tile([128, KT, 128], F32, tag="a")
        xp = sb.tile([128, KT, 128], F16)

        def pool_mm(jp):
            for h in range(2):
                j = 2 * jp + h
                for t in range(KT):
                    nc.tensor.matmul(xpps[:, t, 4 * j:4 * j + 4],
                                     xt[:, j, bass.ts(t, 128)], inds,
                                     start=True, stop=True)
            if jp % 2 == 1:
                ch = (jp - 1) // 2
                if ch == 5:
                    # last pooled block gates H: split the copy
                    nc.scalar.copy(xp[:, :, 80:88], xpps[:, :, 80:88])
                    nc.vector.tensor_copy(xp[:, :, 88:96], xpps[:, :, 88:96])
                elif ch % 2 == 0:
                    nc.scalar.copy(xp[:, :, 16 * ch:16 * ch + 16],
                                   xpps[:, :, 16 * ch:16 * ch + 16])
                else:
                    nc.vector.tensor_copy(xp[:, :, 16 * ch:16 * ch + 16],
                                          xpps[:, :, 16 * ch:16 * ch + 16])

        # chunks 6 and 7 pool on DVE straight from x_sb (short post-arrival
        # chain); their halves arrive first so the four serial DVE reduces
        # hide inside the x stream. Their x.T tiles still come from the PE
        # transposes (only M1, much later, needs them).
        for hh in (12, 13, 14, 15):
            nc.vector.reduce_sum(
                out=xp[:, :, 8 * hh:8 * hh + 8],
                in_=x_sb[:, :, 256 * hh:256 * hh + 256].rearrange(
                    "p k (s t) -> p k s t", t=SEG),
                axis=mybir.AxisListType.X)
        NPAIR = NLT // 2   # 16 pairs, all chunks transposed
        T_ORDER = [12, 13, 14, 15] + list(range(12))
        for idx, jp in enumerate(T_ORDER):
            tp = psB.tile([128, 2, 512], F16, tag="b")
            for h in range(2):
                j = 2 * jp + h
                for k in range(KT):
                    nc.tensor.transpose(tp[:, h, bass.ts(k, 128)],
                                        x_sb[:, k, bass.ts(j, 128)], ident)
            if jp == 11:
                # the last pooled pair gates H: split its copy
                nc.vector.tensor_copy(xt[:, 22:23, :], tp[:, 0:1])
                nc.scalar.copy(xt[:, 23:24, :], tp[:, 1:2])
            elif jp % 2 == 0:
                # alternate engines: neither alone sustains the 728ns
                # x-half cadence once per-op overheads are counted
                nc.vector.tensor_copy(xt[:, 2 * jp:2 * jp + 2, :], tp)
            else:
                nc.scalar.copy(xt[:, 2 * jp:2 * jp + 2, :], tp)
            if idx >= 2 and T_ORDER[idx - 2] < 12:
                pool_mm(T_ORDER[idx - 2])
        pool_mm(10)
        pool_mm(11)

        # ---- H, G, hq (biases folded into psum as K=1 matmuls) ---------------
        # H lands in two psum tiles so the first half's copy-out (which
        # S1's k=0/1 matmuls wait on) does not WAR-block the second half's
        # matmuls on the same tile.
        h_sb = sb.tile([128, KT, 128], F16)
        for half in range(2):
            hps = psA.tile([128, 2, 128], F32, tag="a")
            for eh in range(2):
                et = 2 * half + eh
                for ck in range(KT):
                    nc.tensor.matmul(hps[:, eh, :],
                                     mks[:, ck, bass.ts(et, 128)],
                                     xp[:, ck, :],
                                     start=(ck == 0), stop=False)
                nc.tensor.matmul(hps[:, eh, :],
                                 vkr[:, bass.ts(et, 128)], onesr,
                                 start=False, stop=True)
            if half == 0:
                nc.scalar.copy(h_sb[:, 0:2, :], hps)
            else:
                nc.vector.tensor_copy(h_sb[:, 2:4, :], hps)
        g_sb = sb.tile([128, KT, 128], F16)
        hq = sb.tile([128, 1], F32)

        def emit_g():
            # deferred past s1_stage(0): G waits on the late mqs DMA and
            # must not gate S1 through PE program order
            gps = psA.tile([128, KT, 128], F32, tag="a")
            for et in range(KT):
                for ck in range(KT):
                    nc.tensor.matmul(gps[:, et, :],
                                     mqs[:, ck, bass.ts(et, 128)],
                                     xp[:, ck, :],
                                     start=(ck == 0), stop=False)
                nc.tensor.matmul(gps[:, et, :],
                                 vqr[:, bass.ts(et, 128)], onesr,
                                 start=False, stop=True)
            nc.vector.tensor_copy(g_sb, gps)

        def emit_hq():
            hqps = psA.tile([128, 2], F32, tag="a")
            for ck in range(KT):
                nc.tensor.matmul(hqps, xp[:, ck, :], vq2[:, ck, :],
                                 start=(ck == 0), stop=False)
            nc.tensor.matmul(hqps, onesr, hb2, start=False, stop=True)
            nc.vector.tensor_copy(hq, hqps[:, 0:1])

        # ---- pass 1: S1 -> E1 -> (xbar) E1T -> M1, software-pipelined --------
        # M1 for chunk a is issued after S1 for chunk a+3, so the PE never
        # stalls on the exp + xbar-transpose round trip.
        rs1 = sb.tile([128, NCH], F32)
        m1tps = []
        for ek in range(KT):
            m1b = psB.tile([128, 128], F32, tag="b")
            m1tps.append(m1b)
        e1ts = []
        e2s = []
        eng_ns = [0.0, 0.0]          # accumulated Act / DVE pass-2 time
        SCALE_COST = (612.0, 658.0)  # Act identity-scale / DVE tsp mul

        def s1_stage(a):
            s1 = psA.tile([128, 512], F32, tag="a")
            for k in range(KT):
                nc.tensor.matmul(s1, h_sb[:, k, :], x_sb[:, k, bass.ts(a, 512)],
                                 start=(k == 0), stop=(k == KT - 1))
            e1 = e1p.tile([128, 512], BF16, tag="e1")
            nc.scalar.activation(out=e1, in_=s1,
                                 func=mybir.ActivationFunctionType.Exp,
                                 bias=sh1, scale=1.0,
                                 accum_out=rs1[:, a:a + 1])
            e1t = e1tp.tile([128, 4, 128], BF16, tag="e1t")
            nc.sync.dma_start_transpose(e1t, e1)
            e1ts.append(e1t)

        def s2_stage(a):
            s2 = psA.tile([128, 512], F32, tag="a")
            for k in range(KT):
                nc.tensor.matmul(s2, g_sb[:, k, :], x_sb[:, k, bass.ts(a, 512)],
                                 start=(k == 0), stop=(k == KT - 1))
            e2 = e2p.tile([128, 512], BF16, tag="e2")
            nc.scalar.activation(out=e2, in_=s2,
                                 func=mybir.ActivationFunctionType.Exp,
                                 bias=hq, scale=1.0)
            eng_ns[0] += 611.0
            e2s.append(e2)

        def m1_stage(a, split_last=False):
            e1t = e1ts[a]
            if not split_last:
                for u in range(4):
                    j = 4 * a + u
                    for ek in range(KT):
                        nc.tensor.matmul(m1tps[ek], xt[:, j, bass.ts(ek, 128)],
                                         e1t[:, u, :],
                                         start=(j == 0), stop=False)
            else:
                # bank-major order so each m1t bank closes (and can be
                # copied out) while PE still works on the next bank
                for ek in range(KT):
                    for u in range(4):
                        j = 4 * a + u
                        nc.tensor.matmul(m1tps[ek], xt[:, j, bass.ts(ek, 128)],
                                         e1t[:, u, :],
                                         start=False, stop=(u == 3))
                    if ek % 2 == 0:
                        nc.scalar.copy(m1t[:, ek, :], m1tps[ek])
                    else:
                        nc.vector.tensor_copy(m1t[:, ek, :], m1tps[ek])

        m1t = sb.tile([128, KT, 128], BF16)
        s1_stage(0)
        emit_g()
        s1_stage(1)
        emit_hq()
        for a in range(2, NCH):
            s1_stage(a)
        s2_stage(0)
        s2_stage(1)
        for a in range(NCH - 1):
            m1_stage(a)
            if a == 1:
                rsum1 = sb.tile([128, 1], F32)
                nc.vector.reduce_sum(out=rsum1, in_=rs1,
                                     axis=mybir.AxisListType.X)
                inv1 = sb.tile([128, 1], F32)
                nc.vector.reciprocal(inv1, rsum1)
        m1_stage(NCH - 1, split_last=True)

        # ---- c = (M1 @ Wv^T)/rowsum1 + bv ------------------------------------
        cps = psA.tile([128, D], F32, tag="a")
        for i in range(KT):
            nc.tensor.matmul(cps, m1t[:, i, :], wvt[:, i, :],
                             start=(i == 0), stop=(i == KT - 1))
        cbv = sb.tile([128, D], BF16)
        nc.vector.scalar_tensor_tensor(out=cbv, in0=cps, scalar=inv1, in1=bvb,
                                       op0=mybir.AluOpType.mult,
                                       op1=mybir.AluOpType.add)

        # ---- pass 2: S2 -> E2 -> out = (E2^T cbv) * inv2, software-pipelined -
        or_ = OUT.rearrange("(c j p) d -> c p j d", j=4, p=128)
        inv2s = []

        def rs2_stage(a):
            e2 = e2s[a]
            rsps = psB.tile([128, 4, 2], F32, tag="b")
            for u in range(4):
                nc.tensor.matmul(rsps[:, u, :], e2[:, bass.ts(u, 128)], ones2,
                                 start=True, stop=True)
            inv2 = iv2p.tile([128, 4], F32, tag="iv2")
            nc.vector.reciprocal(inv2, rsps[:, :, 0])
            eng_ns[1] += 129.0
            inv2s.append(inv2)

        def out_stage(a):
            e2 = e2s[a]
            inv2 = inv2s[a]
            last = a == NCH - 1
            o_sb = outp.tile([128, 4, D], F16, tag="o")
            for u in range(4):
                # odd u-tiles borrow psB so neither psum ring becomes the
                # per-chunk bottleneck (psB otherwise only holds tiny rsps)
                pool, ptag = (psA, "a") if u % 2 == 0 else (psB, "b")
                ops = pool.tile([128, D], F32, tag=ptag)
                nc.tensor.matmul(ops, e2[:, bass.ts(u, 128)], cbv,
                                 start=True, stop=True)
                if last:
                    eng = 1 if u % 2 == 0 else 0
                else:
                    eng = min((0, 1),
                              key=lambda i: eng_ns[i] + SCALE_COST[i])
                eng_ns[eng] += SCALE_COST[eng]
                if eng == 0:
                    nc.scalar.activation(
                        out=o_sb[:, u, :], in_=ops,
                        func=mybir.ActivationFunctionType.Identity,
                        bias=0.0, scale=inv2[:, u:u + 1])
                else:
                    nc.vector.tensor_scalar_mul(o_sb[:, u, :], ops,
                                                inv2[:, u:u + 1])
                if u == 1:
                    nc.sync.dma_start(out=or_[a][:, 0:2, :],
                                      in_=o_sb[:, 0:2, :])
            nc.sync.dma_start(out=or_[a][:, 2:4, :], in_=o_sb[:, 2:4, :])

        rs2_stage(0)
        for a in range(NCH):
            if a + 2 < NCH:
                s2_stage(a + 2)
            if a + 1 < NCH:
                rs2_stage(a + 1)
            out_stage(a)

    nc.compile()
    return nc


def _host_inputs(x, Wq, bq, Wk, bk, Wv, bv):
    del bk  # stage-1 softmax is invariant to the k-projection bias
    Wq = np.asarray(Wq, dtype=np.float32)
    Wk = np.asarray(Wk, dtype=np.float32)
    Wv = np.asarray(Wv, dtype=np.float32)
    bq = np.asarray(bq, dtype=np.float32)
    bv = np.asarray(bv, dtype=np.float32)
    bf16 = ml_dtypes.bfloat16
    inds = np.zeros((128, 4), dtype=np.float32)
    inds[np.arange(128), np.arange(128) // SEG] = 1.0
    ones2 = np.zeros((128, 2), dtype=np.float32)
    ones2[:, 0] = 1.0
    common = {
        "mks": ((Wq.T @ Wk) / SEG).astype(np.float16),
        "mqs": ((Wq.T @ Wq) / SEG).astype(np.float16),
        "wvt": np.ascontiguousarray(Wv.T).astype(bf16),
        "ident": np.eye(128, dtype=np.float16),
        "inds": inds.astype(bf16),
        "ones2": ones2.astype(bf16),
        "vkr": (Wk.T @ bq).astype(np.float16),
        "vqr": (Wq.T @ bq).astype(np.float16),
        "vq2": np.stack([(Wq.T @ bq) / SEG, np.zeros(D, np.float32)],
                        axis=1).astype(np.float16),
        "hb2": np.array([float(bq @ bq) - SHIFT2, 0.0], dtype=np.float16),
        "onesr": np.ones(128, dtype=np.float16),
        "bvb": np.tile(bv[None, :], (128, 1)).astype(bf16),
    }
    maps = []
    for b in range(B):
        m = dict(common)
        m["x"] = np.ascontiguousarray(x[b]).astype(np.float16)
        maps.append(m)
    return maps


def kernel(x, Wq, bq, Wk, bk, Wv, bv):
    x = np.asarray(x, dtype=np.float32)
    if "nc" not in _CACHE:
        _CACHE["nc"] = build()
    nc = _CACHE["nc"]
    in_maps = _host_inputs(x, Wq, bq, Wk, bk, Wv, bv)
    res = run_bass_kernel_spmd(nc, in_maps, core_ids=list(range(B)))
    out = np.empty((B, D, L), dtype=np.float32)
    for b in range(B):
        out[b] = np.asarray(res.results[b]["out"]).astype(np.float32).T
    return out


# revision 48
# speedup vs baseline: 1.0066x; 1.0066x over previous
"""AgentSelfAttention1d Trainium2 kernel (v3).

Per batch b (one NeuronCore each):
    xt = x[b].T                       # [L=4096, D=512]
    q/k/v = xt @ W{q,k,v}.T + b       # [L, D]
    a  = AdaptiveAvgPool(q) -> [P=128, D]
    c  = softmax(a @ k.T, -1) @ v     # [P, D]
    r  = softmax(q @ a.T, -1) @ c     # [L, D]
    out[b] = r.T                      # [D, L]

Restructuring (all projections folded into host-precomputed weight
products; everything channel-first on chip):
    xp[c,p]   = seg-sum of x over 32-wide windows      (tiny PE matmuls
                against a one-hot segment indicator, from the x.T tiles)
    H[e,p]    = MKs[c,e]^T-contract xp + vk[e],  MKs = (Wq^T Wk)/32,
                vk = Wk^T bq          (S1[p,l] = sum_e H[e,p] x[e,l])
    G[e,p]    = MQs-contract xp + vq[e],         MQs = (Wq^T Wq)/32
    hq[p]     = (xp^T (Wq^T bq))/32 + |bq|^2     (S2[p,l] = G-part + hq)
    E1        = exp(S1 - 10)  bf16; rowsum via activation accumulator
    E2        = exp(S2 - 40)  bf16 (unnormalized; bf16 keeps the f32
                exponent range so all-underflow columns cannot occur)
    M1T[e,p]  = (E1 @ x.T)^T, accumulated directly transposed across four
                PSUM banks; E1 transposed by the DMA xbar engine
    cbv[p,d]  = (M1 @ Wv^T) / rowsum1 + bv
    out[l,d]  = (sum_p E2[p,l] cbv[p,d]) / colsum2[l]
    Output written [L, D] fp16; host transposes/upcasts to [D, L] f32.

v3 schedule: x DMAs are issued first (ident ahead of them on the sync
queue) so transposes start ~2.5us in; all 8 chunks are PE-transposed and
pooled uniformly during the x-load window (no DVE/xbar special case for
chunk 7, which previously sat on the critical path after the last x
chunk); pass 1 interleaves S1 and S2 per chunk (E1 and E2 exps spread
across the whole pass on the Act engine) with M1 trailing two chunks;
pass 2 is only the 32 output matmuls with colsum2 (rs2) one chunk ahead,
scales round-robined over Act/DVE/GpSimd, and the final chunk stored as
four small DMAs to cut the tail.
"""

import numpy as np
import ml_dtypes

import concourse.bass as bass
import concourse.mybir as mybir
import concourse.tile as tile
from concourse import bacc
from concourse.bass_utils import run_bass_kernel_spmd

F32 = mybir.dt.float32
F16 = mybir.dt.float16
BF16 = mybir.dt.bfloat16

B, D, L, P = 8, 512, 4096, 128
KT = D // 128      # 4 contraction tiles of 128
NCH = L // 512     # 8 l-chunks of 512
NLT = L // 128     # 32 l-tiles of 128
SEG = L // P       # 32: pool segment length
SHIFT1 = 10.0
SHIFT2 = 40.0

_CACHE = {}


def build():
    nc = bacc.Bacc(target_bir_lowering=False, trn_type="TRN2")
    X = nc.dram_tensor("x", [D, L], F16, kind="ExternalInput")
    MKS = nc.dram_tensor("mks", [D, D], F16, kind="ExternalInput")   # (Wq^T Wk)/32 [c,e]
    MQS = nc.dram_tensor("mqs", [D, D], F16, kind="ExternalInput")   # (Wq^T Wq)/32 [c,e]
    WVT = nc.dram_tensor("wvt", [D, D], BF16, kind="ExternalInput")  # Wv^T [e,d]
    IDN = nc.dram_tensor("ident", [128, 128], F16, kind="ExternalInput")
    INDS = nc.dram_tensor("inds", [128, 4], BF16, kind="ExternalInput")  # l -> l//32 one-hot
    ONES2 = nc.dram_tensor("ones2", [128, 2], BF16, kind="ExternalInput")
    VKR = nc.dram_tensor("vkr", [D], F16, kind="ExternalInput")      # Wk^T bq row
    VQR = nc.dram_tensor("vqr", [D], F16, kind="ExternalInput")      # Wq^T bq row
    VQ2 = nc.dram_tensor("vq2", [D, 2], F16, kind="ExternalInput")   # [(Wq^T bq)/32, 0]
    HB2 = nc.dram_tensor("hb2", [2], F16, kind="ExternalInput")      # [|bq|^2-S2, 0]
    ONESR = nc.dram_tensor("onesr", [128], F16, kind="ExternalInput")
    BVB = nc.dram_tensor("bvb", [128, D], BF16, kind="ExternalInput")     # bv bcast
    OUT = nc.dram_tensor("out", [L, D], F16, kind="ExternalOutput")

    from contextlib import ExitStack
    with nc.allow_low_precision("16-bit matmul operands"), \
         tile.TileContext(nc, pool_alloc_mode="queue") as tc, ExitStack() as stack:
        sb = stack.enter_context(tc.tile_pool(name="sb", bufs=1))
        e1p = stack.enter_context(tc.tile_pool(name="e1p", bufs=8))
        e1tp = stack.enter_context(tc.tile_pool(name="e1tp", bufs=8))
        e2p = stack.enter_context(tc.tile_pool(name="e2p", bufs=8))
        outp = stack.enter_context(tc.tile_pool(name="outp", bufs=8))
        iv2p = stack.enter_context(tc.tile_pool(name="iv2p", bufs=8))
        # PSUM (8 banks): load: xpps 1 + tp ring | prep: h/g/hq ring |
        # pass1: s1+s2 ring 4 (psA) + m1t 4 (psB) | pass2: out 4 + rs2
        psA = stack.enter_context(tc.tile_pool(name="psA", bufs=4, space="PSUM"))
        psB = stack.enter_context(tc.tile_pool(name="psB", bufs=4, space="PSUM"))

        # ---- ACT table warmup ------------------------------------------------
        warm = sb.tile([128, 1], F32)
        nc.vector.memset(warm, 0.0)
        nc.scalar.activation(out=warm, in_=warm,
                             func=mybir.ActivationFunctionType.Exp,
                             bias=warm, scale=1.0)

        # ---- input DMAs: x first on sync; ident leads the SWDGE queue --------
        ident = sb.tile([128, 128], F16)
        nc.gpsimd.dma_start(out=ident, in_=IDN[:, :])
        x_sb = sb.tile([128, KT, L], F16)
        xr = X.rearrange("(k p) l -> p k l", p=128)
        # halves 12-15 (chunks 6,7) pool via DVE reduces, which serialize on
        # DVE: spread them through the stream so each reduce hides inside
        # the x window without compressing the pool_mm chunks' arrivals.
        # mks arrives after x with a near-zero post-arrival chain.
        H_ORDER = [14, 0, 1, 2, 15, 3, 4, 5, 12, 6, 7, 8, 13, 9, 10, 11]
        for hch in H_ORDER:
            nc.sync.dma_start(out=x_sb[:, :, bass.ts(hch, 256)],
                              in_=xr[:, :, bass.ts(hch, 256)])
        # small tensors ride the SWDGE queue (prep overlaps the x stream)
        inds = sb.tile([128, 4], BF16)
        nc.gpsimd.dma_start(out=inds, in_=INDS[:, :])
        vkr = sb.tile([1, D], F16)
        nc.gpsimd.dma_start(out=vkr, in_=VKR.rearrange("(o d) -> o d", o=1))
        vqr = sb.tile([1, D], F16)
        nc.gpsimd.dma_start(out=vqr, in_=VQR.rearrange("(o d) -> o d", o=1))
        vq2 = sb.tile([128, KT, 2], F16)
        nc.gpsimd.dma_start(out=vq2, in_=VQ2.rearrange("(k p) t -> p k t", p=128))
        hb2 = sb.tile([1, 2], F16)
        nc.gpsimd.dma_start(out=hb2, in_=HB2.rearrange("(o d) -> o d", o=1))
        onesr = sb.tile([1, 128], F16)
        nc.gpsimd.dma_start(out=onesr, in_=ONESR.rearrange("(o d) -> o d", o=1))
        # weights after x on the sync queue; mks in two halves so H's first
        # contraction pair starts one transfer earlier
        mks = sb.tile([128, KT, D], F16)
        mksr = MKS.rearrange("(k p) e -> p k e", p=128)
        nc.sync.dma_start(out=mks[:, 0:2, :], in_=mksr[:, 0:2, :])
        nc.sync.dma_start(out=mks[:, 2:4, :], in_=mksr[:, 2:4, :])
        mqs = sb.tile([128, KT, D], F16)
        nc.sync.dma_start(out=mqs, in_=MQS.rearrange("(k p) e -> p k e", p=128))
        wvt = sb.tile([128, KT, D], BF16)
        nc.sync.dma_start(out=wvt, in_=WVT.rearrange("(k p) e -> p k e", p=128))
        ones2 = sb.tile([128, 2], BF16)
        nc.sync.dma_start(out=ones2, in_=ONES2[:, :])
        bvb = sb.tile([128, D], BF16)
        nc.sync.dma_start(out=bvb, in_=BVB[:, :])
        sh1 = sb.tile([128, 1], F32)
        nc.vector.memset(sh1, -SHIFT1)

        # ---- x.T tiles (PE transpose) + pooling (tiny PE matmuls) ------------
        # All 8 chunks uniformly; runs inside the x-load window where PE,
        # Act, DVE and GpSimd are otherwise idle.
        xt = sb.tile([128, NLT, D], BF16)
        xpps = psA.tile([128, KT, 128], F32, tag="a")
        xp = sb.tile([128, KT, 128], F16)

        def pool_mm(jp):
            for h in range(2):
                j = 2 * jp + h
                for t in range(KT):
                    nc.tensor.matmul(xpps[:, t, 4 * j:4 * j + 4],
                                     xt[:, j, bass.ts(t, 128)], inds,
                                     start=True, stop=True)
            if jp % 2 == 1:
                ch = (jp - 1) // 2
                if ch == 5:
                    # last pooled block gates H: split the copy
                    nc.scalar.copy(xp[:, :, 80:88], xpps[:, :, 80:88])
                    nc.vector.tensor_copy(xp[:, :, 88:96], xpps[:, :, 88:96])
                elif ch % 2 == 0:
                    nc.scalar.copy(xp[:, :, 16 * ch:16 * ch + 16],
                                   xpps[:, :, 16 * ch:16 * ch + 16])
                else:
                    nc.vector.tensor_copy(xp[:, :, 16 * ch:16 * ch + 16],
                                          xpps[:, :, 16 * ch:16 * ch + 16])

        # chunks 6 and 7 pool on DVE straight from x_sb (short post-arrival
        # chain); their halves arrive first so the four serial DVE reduces
        # hide inside the x stream. Their x.T tiles still come from the PE
        # transposes (only M1, much later, needs them).
        for hh in (12, 13, 14, 15):
            nc.vector.reduce_sum(
                out=xp[:, :, 8 * hh:8 * hh + 8],
                in_=x_sb[:, :, 256 * hh:256 * hh + 256].rearrange(
                    "p k (s t) -> p k s t", t=SEG),
                axis=mybir.AxisListType.X)
        NPAIR = NLT // 2   # 16 pairs, all chunks transposed
        T_ORDER = H_ORDER
        for idx, jp in enumerate(T_ORDER):
            tp = psB.tile([128, 2, 512], F16, tag="b")
            for h in range(2):
                j = 2 * jp + h
                for k in range(KT):
                    nc.tensor.transpose(tp[:, h, bass.ts(k, 128)],
                                        x_sb[:, k, bass.ts(j, 128)], ident)
            if jp == 11:
                # the last pooled pair gates H: split its copy
                nc.vector.tensor_copy(xt[:, 22:23, :], tp[:, 0:1])
                nc.scalar.copy(xt[:, 23:24, :], tp[:, 1:2])
            elif jp % 2 == 0:
                # alternate engines: neither alone sustains the 728ns
                # x-half cadence once per-op overheads are counted
                nc.vector.tensor_copy(xt[:, 2 * jp:2 * jp + 2, :], tp)
            else:
                nc.scalar.copy(xt[:, 2 * jp:2 * jp + 2, :], tp)
            if idx >= 2 and T_ORDER[idx - 2] < 12:
                pool_mm(T_ORDER[idx - 2])
        pool_mm(10)
        pool_mm(11)

        # ---- H, G, hq (biases folded into psum as K=1 matmuls) ---------------
        # H lands in two psum tiles so the first half's copy-out (which
        # S1's k=0/1 matmuls wait on) does not WAR-block the second half's
        # matmuls on the same tile.
        h_sb = sb.tile([128, KT, 128], F16)
        for half in range(2):
            hps = psA.tile([128, 2, 128], F32, tag="a")
            # ck-major: the ck 0/1 matmuls depend only on the first mks DMA
            for ck in range(KT):
                for eh in range(2):
                    et = 2 * half + eh
                    nc.tensor.matmul(hps[:, eh, :],
                                     mks[:, ck, bass.ts(et, 128)],
                                     xp[:, ck, :],
                                     start=(ck == 0), stop=False)
            for eh in range(2):
                et = 2 * half + eh
                nc.tensor.matmul(hps[:, eh, :],
                                 vkr[:, bass.ts(et, 128)], onesr,
                                 start=False, stop=True)
            if half == 0:
                nc.scalar.copy(h_sb[:, 0:2, :], hps)
            else:
                nc.vector.tensor_copy(h_sb[:, 2:4, :], hps)
        g_sb = sb.tile([128, KT, 128], F16)
        hq = sb.tile([128, 1], F32)

        def emit_g():
            # deferred past s1_stage(0): G waits on the late mqs DMA and
            # must not gate S1 through PE program order
            gps = psA.tile([128, KT, 128], F32, tag="a")
            for et in range(KT):
                for ck in range(KT):
                    nc.tensor.matmul(gps[:, et, :],
                                     mqs[:, ck, bass.ts(et, 128)],
                                     xp[:, ck, :],
                                     start=(ck == 0), stop=False)
                nc.tensor.matmul(gps[:, et, :],
                                 vqr[:, bass.ts(et, 128)], onesr,
                                 start=False, stop=True)
            nc.vector.tensor_copy(g_sb, gps)

        def emit_hq():
            hqps = psA.tile([128, 2], F32, tag="a")
            for ck in range(KT):
                nc.tensor.matmul(hqps, xp[:, ck, :], vq2[:, ck, :],
                                 start=(ck == 0), stop=False)
            nc.tensor.matmul(hqps, onesr, hb2, start=False, stop=True)
            nc.vector.tensor_copy(hq, hqps[:, 0:1])

        # ---- pass 1: S1 -> E1 -> (xbar) E1T -> M1, software-pipelined --------
        # M1 for chunk a is issued after S1 for chunk a+3, so the PE never
        # stalls on the exp + xbar-transpose round trip.
        rs1 = sb.tile([128, NCH], F32)
        m1tps = []
        for ek in range(KT):
            m1b = psB.tile([128, 128], F32, tag="b")
            m1tps.append(m1b)
        e1ts = []
        e2s = []
        eng_ns = [0.0, 0.0]          # accumulated Act / DVE pass-2 time
        SCALE_COST = (612.0, 658.0)  # Act identity-scale / DVE tsp mul

        def s1_stage(a):
            s1 = psA.tile([128, 512], F32, tag="a")
            for k in range(KT):
                nc.tensor.matmul(s1, h_sb[:, k, :], x_sb[:, k, bass.ts(a, 512)],
                                 start=(k == 0), stop=(k == KT - 1))
            e1 = e1p.tile([128, 512], BF16, tag="e1")
            nc.scalar.activation(out=e1, in_=s1,
                                 func=mybir.ActivationFunctionType.Exp,
                                 bias=sh1, scale=1.0,
                                 accum_out=rs1[:, a:a + 1])
            e1t = e1tp.tile([128, 4, 128], BF16, tag="e1t")
            nc.sync.dma_start_transpose(e1t, e1)
            e1ts.append(e1t)

        def s2_stage(a):
            s2 = psA.tile([128, 512], F32, tag="a")
            for k in range(KT):
                nc.tensor.matmul(s2, g_sb[:, k, :], x_sb[:, k, bass.ts(a, 512)],
                                 start=(k == 0), stop=(k == KT - 1))
            e2 = e2p.tile([128, 512], BF16, tag="e2")
            nc.scalar.activation(out=e2, in_=s2,
                                 func=mybir.ActivationFunctionType.Exp,
                                 bias=hq, scale=1.0)
            eng_ns[0] += 611.0
            e2s.append(e2)

        def m1_stage(a, split_last=False):
            e1t = e1ts[a]
            if not split_last:
                for u in range(4):
                    j = 4 * a + u
                    for ek in range(KT):
                        nc.tensor.matmul(m1tps[ek], xt[:, j, bass.ts(ek, 128)],
                                         e1t[:, u, :],
                                         start=(j == 0), stop=False)
            else:
                # bank-major order so each m1t bank closes (and can be
                # copied out) while PE still works on the next bank
                for ek in range(KT):
                    for u in range(4):
                        j = 4 * a + u
                        nc.tensor.matmul(m1tps[ek], xt[:, j, bass.ts(ek, 128)],
                                         e1t[:, u, :],
                                         start=False, stop=(u == 3))
                    if ek % 2 == 0:
                        nc.scalar.copy(m1t[:, ek, :], m1tps[ek])
                    else:
                        nc.vector.tensor_copy(m1t[:, ek, :], m1tps[ek])

        m1t = sb.tile([128, KT, 128], BF16)
        s1_stage(0)
        emit_g()
        s1_stage(1)
        emit_hq()
        for a in range(2, NCH):
            s1_stage(a)
        s2_stage(0)
        s2_stage(1)
        for a in range(NCH - 1):
            m1_stage(a)
            if a == 1:
                rsum1 = sb.tile([128, 1], F32)
                nc.vector.reduce_sum(out=rsum1, in_=rs1,
                                     axis=mybir.AxisListType.X)
                inv1 = sb.tile([128, 1], F32)
                nc.vector.reciprocal(inv1, rsum1)
        m1_stage(NCH - 1, split_last=True)

        # ---- c = (M1 @ Wv^T)/rowsum1 + bv ------------------------------------
        cps = psA.tile([128, D], F32, tag="a")
        for i in range(KT):
            nc.tensor.matmul(cps, m1t[:, i, :], wvt[:, i, :],
                             start=(i == 0), stop=(i == KT - 1))
        cbv = sb.tile([128, D], BF16)
        nc.vector.scalar_tensor_tensor(out=cbv, in0=cps, scalar=inv1, in1=bvb,
                                       op0=mybir.AluOpType.mult,
                                       op1=mybir.AluOpType.add)

        # ---- pass 2: S2 -> E2 -> out = (E2^T cbv) * inv2, software-pipelined -
        or_ = OUT.rearrange("(c j p) d -> c p j d", j=4, p=128)
        inv2s = []

        def rs2_stage(a):
            e2 = e2s[a]
            rsps = psB.tile([128, 4, 2], F32, tag="b")
            for u in range(4):
                nc.tensor.matmul(rsps[:, u, :], e2[:, bass.ts(u, 128)], ones2,
                                 start=True, stop=True)
            inv2 = iv2p.tile([128, 4], F32, tag="iv2")
            nc.vector.reciprocal(inv2, rsps[:, :, 0])
            eng_ns[1] += 129.0
            inv2s.append(inv2)

        def out_stage(a):
            e2 = e2s[a]
            inv2 = inv2s[a]
            last = a == NCH - 1
            o_sb = outp.tile([128, 4, D], F16, tag="o")
            for u in range(4):
                # odd u-tiles borrow psB so neither psum ring becomes the
                # per-chunk bottleneck (psB otherwise only holds tiny rsps)
                pool, ptag = (psA, "a") if u % 2 == 0 else (psB, "b")
                ops = pool.tile([128, D], F32, tag=ptag)
                nc.tensor.matmul(ops, e2[:, bass.ts(u, 128)], cbv,
                                 start=True, stop=True)
                if last:
                    eng = 1 if u % 2 == 0 else 0
                else:
                    eng = min((0, 1),
                              key=lambda i: eng_ns[i] + SCALE_COST[i])
                eng_ns[eng] += SCALE_COST[eng]
                if eng == 0:
                    nc.scalar.activation(
                        out=o_sb[:, u, :], in_=ops,
                        func=mybir.ActivationFunctionType.Identity,
                        bias=0.0, scale=inv2[:, u:u + 1])
                else:
                    nc.vector.tensor_scalar_mul(o_sb[:, u, :], ops,
                                                inv2[:, u:u + 1])
                if u == 1:
                    nc.sync.dma_start(out=or_[a][:, 0:2, :],
                                      in_=o_sb[:, 0:2, :])
            nc.sync.dma_start(out=or_[a][:, 2:4, :], in_=o_sb[:, 2:4, :])

        rs2_stage(0)
        for a in range(NCH):
            if a + 2 < NCH:
                s2_stage(a + 2)
            if a + 1 < NCH:
                rs2_stage(a + 1)
            out_stage(a)

    nc.compile()
    return nc


def _host_inputs(x, Wq, bq, Wk, bk, Wv, bv):
    del bk  # stage-1 softmax is invariant to the k-projection bias
    Wq = np.asarray(Wq, dtype=np.float32)
    Wk = np.asarray(Wk, dtype=np.float32)
    Wv = np.asarray(Wv, dtype=np.float32)
    bq = np.asarray(bq, dtype=np.float32)
    bv = np.asarray(bv, dtype=np.float32)
    bf16 = ml_dtypes.bfloat16
    inds = np.zeros((128, 4), dtype=np.float32)
    inds[np.arange(128), np.arange(128) // SEG] = 1.0
    ones2 = np.zeros((128, 2), dtype=np.float32)
    ones2[:, 0] = 1.0
    common = {
        "mks": ((Wq.T @ Wk) / SEG).astype(np.float16),
        "mqs": ((Wq.T @ Wq) / SEG).astype(np.float16),
        "wvt": np.ascontiguousarray(Wv.T).astype(bf16),
        "ident": np.eye(128, dtype=np.float16),
        "inds": inds.astype(bf16),
        "ones2": ones2.astype(bf16),
        "vkr": (Wk.T @ bq).astype(np.float16),
        "vqr": (Wq.T @ bq).astype(np.float16),
        "vq2": np.stack([(Wq.T @ bq) / SEG, np.zeros(D, np.float32)],
                        axis=1).astype(np.float16),
        "hb2": np.array([float(bq @ bq) - SHIFT2, 0.0], dtype=np.float16),
        "onesr": np.ones(128, dtype=np.float16),
        "bvb": np.tile(bv[None, :], (128, 1)).astype(bf16),
    }
    maps = []
    for b in range(B):
        m = dict(common)
        m["x"] = np.ascontiguousarray(x[b]).astype(np.float16)
        maps.append(m)
    return maps


def kernel(x, Wq, bq, Wk, bk, Wv, bv):
    x = np.asarray(x, dtype=np.float32)
    if "nc" not in _CACHE:
        _CACHE["nc"] = build()
    nc = _CACHE["nc"]
    in_maps = _host_inputs(x, Wq, bq, Wk, bk, Wv, bv)
    res = run_bass_kernel_spmd(nc, in_maps, core_ids=list(range(B)))
    out = np.empty((B, D, L), dtype=np.float32)
    for b in range(B):
        out[b] = np.asarray(res.results[b]["out"]).astype(np.float32).T
    return out


# revision 49
# speedup vs baseline: 1.0114x; 1.0048x over previous
"""AgentSelfAttention1d Trainium2 kernel (v3).

Per batch b (one NeuronCore each):
    xt = x[b].T                       # [L=4096, D=512]
    q/k/v = xt @ W{q,k,v}.T + b       # [L, D]
    a  = AdaptiveAvgPool(q) -> [P=128, D]
    c  = softmax(a @ k.T, -1) @ v     # [P, D]
    r  = softmax(q @ a.T, -1) @ c     # [L, D]
    out[b] = r.T                      # [D, L]

Restructuring (all projections folded into host-precomputed weight
products; everything channel-first on chip):
    xp[c,p]   = seg-sum of x over 32-wide windows      (tiny PE matmuls
                against a one-hot segment indicator, from the x.T tiles)
    H[e,p]    = MKs[c,e]^T-contract xp + vk[e],  MKs = (Wq^T Wk)/32,
                vk = Wk^T bq          (S1[p,l] = sum_e H[e,p] x[e,l])
    G[e,p]    = MQs-contract xp + vq[e],         MQs = (Wq^T Wq)/32
    hq[p]     = (xp^T (Wq^T bq))/32 + |bq|^2     (S2[p,l] = G-part + hq)
    E1        = exp(S1 - 10)  bf16; rowsum via activation accumulator
    E2        = exp(S2 - 40)  bf16 (unnormalized; bf16 keeps the f32
                exponent range so all-underflow columns cannot occur)
    M1T[e,p]  = (E1 @ x.T)^T, accumulated directly transposed across four
                PSUM banks; E1 transposed by the DMA xbar engine
    cbv[p,d]  = (M1 @ Wv^T) / rowsum1 + bv
    out[l,d]  = (sum_p E2[p,l] cbv[p,d]) / colsum2[l]
    Output written [L, D] fp16; host transposes/upcasts to [D, L] f32.

v3 schedule: x DMAs are issued first (ident ahead of them on the sync
queue) so transposes start ~2.5us in; all 8 chunks are PE-transposed and
pooled uniformly during the x-load window (no DVE/xbar special case for
chunk 7, which previously sat on the critical path after the last x
chunk); pass 1 interleaves S1 and S2 per chunk (E1 and E2 exps spread
across the whole pass on the Act engine) with M1 trailing two chunks;
pass 2 is only the 32 output matmuls with colsum2 (rs2) one chunk ahead,
scales round-robined over Act/DVE/GpSimd, and the final chunk stored as
four small DMAs to cut the tail.
"""

import numpy as np
import ml_dtypes

import concourse.bass as bass
import concourse.mybir as mybir
import concourse.tile as tile
from concourse import bacc
from concourse.bass_utils import run_bass_kernel_spmd

F32 = mybir.dt.float32
F16 = mybir.dt.float16
BF16 = mybir.dt.bfloat16

B, D, L, P = 8, 512, 4096, 128
KT = D // 128      # 4 contraction tiles of 128
NCH = L // 512     # 8 l-chunks of 512
NLT = L // 128     # 32 l-tiles of 128
SEG = L // P       # 32: pool segment length
SHIFT1 = 10.0
SHIFT2 = 40.0

_CACHE = {}


def build():
    nc = bacc.Bacc(target_bir_lowering=False, trn_type="TRN2")
    X = nc.dram_tensor("x", [D, L], F16, kind="ExternalInput")
    MKS = nc.dram_tensor("mks", [D, D], F16, kind="ExternalInput")   # (Wq^T Wk)/32 [c,e]
    MQS = nc.dram_tensor("mqs", [D, D], F16, kind="ExternalInput")   # (Wq^T Wq)/32 [c,e]
    WVT = nc.dram_tensor("wvt", [D, D], BF16, kind="ExternalInput")  # Wv^T [e,d]
    IDN = nc.dram_tensor("ident", [128, 128], F16, kind="ExternalInput")
    INDS = nc.dram_tensor("inds", [128, 4], BF16, kind="ExternalInput")  # l -> l//32 one-hot
    ONES2 = nc.dram_tensor("ones2", [128, 2], BF16, kind="ExternalInput")
    VKR = nc.dram_tensor("vkr", [D], F16, kind="ExternalInput")      # Wk^T bq row
    VQR = nc.dram_tensor("vqr", [D], F16, kind="ExternalInput")      # Wq^T bq row
    VQ2 = nc.dram_tensor("vq2", [D, 2], F16, kind="ExternalInput")   # [(Wq^T bq)/32, 0]
    HB2 = nc.dram_tensor("hb2", [2], F16, kind="ExternalInput")      # [|bq|^2-S2, 0]
    ONESR = nc.dram_tensor("onesr", [128], F16, kind="ExternalInput")
    BVB = nc.dram_tensor("bvb", [128, D], BF16, kind="ExternalInput")     # bv bcast
    OUT = nc.dram_tensor("out", [L, D], F16, kind="ExternalOutput")

    from contextlib import ExitStack
    with nc.allow_low_precision("16-bit matmul operands"), \
         tile.TileContext(nc, pool_alloc_mode="queue") as tc, ExitStack() as stack:
        sb = stack.enter_context(tc.tile_pool(name="sb", bufs=1))
        e1p = stack.enter_context(tc.tile_pool(name="e1p", bufs=8))
        e1tp = stack.enter_context(tc.tile_pool(name="e1tp", bufs=8))
        e2p = stack.enter_context(tc.tile_pool(name="e2p", bufs=8))
        outp = stack.enter_context(tc.tile_pool(name="outp", bufs=8))
        iv2p = stack.enter_context(tc.tile_pool(name="iv2p", bufs=8))
        # PSUM (8 banks): load: xpps 1 + tp ring | prep: h/g/hq ring |
        # pass1: s1+s2 ring 4 (psA) + m1t 4 (psB) | pass2: out 4 + rs2
        psA = stack.enter_context(tc.tile_pool(name="psA", bufs=4, space="PSUM"))
        psB = stack.enter_context(tc.tile_pool(name="psB", bufs=4, space="PSUM"))

        # ---- ACT table warmup ------------------------------------------------
        warm = sb.tile([128, 1], F32)
        nc.vector.memset(warm, 0.0)
        nc.scalar.activation(out=warm, in_=warm,
                             func=mybir.ActivationFunctionType.Exp,
                             bias=warm, scale=1.0)

        # ---- input DMAs: x first on sync; ident leads the SWDGE queue --------
        ident = sb.tile([128, 128], F16)
        nc.gpsimd.dma_start(out=ident, in_=IDN[:, :])
        x_sb = sb.tile([128, KT, L], F16)
        xr = X.rearrange("(k p) l -> p k l", p=128)
        # halves 12-15 (chunks 6,7) pool via DVE reduces, which serialize on
        # DVE: spread them through the stream so each reduce hides inside
        # the x window without compressing the pool_mm chunks' arrivals.
        # mks arrives after x with a near-zero post-arrival chain.
        H_ORDER = [14, 0, 1, 2, 15, 3, 4, 5, 12, 6, 7, 8, 13, 9, 10, 11]
        for hch in H_ORDER:
            nc.sync.dma_start(out=x_sb[:, :, bass.ts(hch, 256)],
                              in_=xr[:, :, bass.ts(hch, 256)])
        # small tensors ride the SWDGE queue (prep overlaps the x stream)
        inds = sb.tile([128, 4], BF16)
        nc.gpsimd.dma_start(out=inds, in_=INDS[:, :])
        vkr = sb.tile([1, D], F16)
        nc.gpsimd.dma_start(out=vkr, in_=VKR.rearrange("(o d) -> o d", o=1))
        vqr = sb.tile([1, D], F16)
        nc.gpsimd.dma_start(out=vqr, in_=VQR.rearrange("(o d) -> o d", o=1))
        vq2 = sb.tile([128, KT, 2], F16)
        nc.gpsimd.dma_start(out=vq2, in_=VQ2.rearrange("(k p) t -> p k t", p=128))
        hb2 = sb.tile([1, 2], F16)
        nc.gpsimd.dma_start(out=hb2, in_=HB2.rearrange("(o d) -> o d", o=1))
        onesr = sb.tile([1, 128], F16)
        nc.gpsimd.dma_start(out=onesr, in_=ONESR.rearrange("(o d) -> o d", o=1))
        # weights after x on the sync queue; mks in two halves so H's first
        # contraction pair starts one transfer earlier
        mks = sb.tile([128, KT, D], F16)
        mksr = MKS.rearrange("(k p) e -> p k e", p=128)
        nc.sync.dma_start(out=mks[:, 0:2, :], in_=mksr[:, 0:2, :])
        nc.sync.dma_start(out=mks[:, 2:4, :], in_=mksr[:, 2:4, :])
        mqs = sb.tile([128, KT, D], F16)
        nc.sync.dma_start(out=mqs, in_=MQS.rearrange("(k p) e -> p k e", p=128))
        wvt = sb.tile([128, KT, D], BF16)
        nc.sync.dma_start(out=wvt, in_=WVT.rearrange("(k p) e -> p k e", p=128))
        ones2 = sb.tile([128, 2], BF16)
        nc.sync.dma_start(out=ones2, in_=ONES2[:, :])
        bvb = sb.tile([128, D], BF16)
        nc.sync.dma_start(out=bvb, in_=BVB[:, :])
        sh1 = sb.tile([128, 1], F32)
        nc.vector.memset(sh1, -SHIFT1)

        # ---- x.T tiles (PE transpose) + pooling (tiny PE matmuls) ------------
        # All 8 chunks uniformly; runs inside the x-load window where PE,
        # Act, DVE and GpSimd are otherwise idle.
        xt = sb.tile([128, NLT, D], BF16)
        xpps = psA.tile([128, KT, 128], F32, tag="a")
        xp = sb.tile([128, KT, 128], F16)

        def pool_mm(jp):
            for h in range(2):
                j = 2 * jp + h
                for t in range(KT):
                    nc.tensor.matmul(xpps[:, t, 4 * j:4 * j + 4],
                                     xt[:, j, bass.ts(t, 128)], inds,
                                     start=True, stop=True)
            if jp % 2 == 1:
                ch = (jp - 1) // 2
                if ch == 5:
                    # last pooled block gates H: split the copy
                    nc.scalar.copy(xp[:, :, 80:88], xpps[:, :, 80:88])
                    nc.vector.tensor_copy(xp[:, :, 88:96], xpps[:, :, 88:96])
                elif ch % 2 == 0:
                    nc.scalar.copy(xp[:, :, 16 * ch:16 * ch + 16],
                                   xpps[:, :, 16 * ch:16 * ch + 16])
                else:
                    nc.vector.tensor_copy(xp[:, :, 16 * ch:16 * ch + 16],
                                          xpps[:, :, 16 * ch:16 * ch + 16])

        # chunks 6 and 7 pool on DVE straight from x_sb (short post-arrival
        # chain); their halves arrive first so the four serial DVE reduces
        # hide inside the x stream. Their x.T tiles still come from the PE
        # transposes (only M1, much later, needs them).
        for hh in (12, 13, 14, 15):
            nc.vector.reduce_sum(
                out=xp[:, :, 8 * hh:8 * hh + 8],
                in_=x_sb[:, :, 256 * hh:256 * hh + 256].rearrange(
                    "p k (s t) -> p k s t", t=SEG),
                axis=mybir.AxisListType.X)
        NPAIR = NLT // 2   # 16 pairs, all chunks transposed
        T_ORDER = H_ORDER
        for idx, jp in enumerate(T_ORDER):
            tp = psB.tile([128, 2, 512], F16, tag="b")
            for h in range(2):
                j = 2 * jp + h
                for k in range(KT):
                    nc.tensor.transpose(tp[:, h, bass.ts(k, 128)],
                                        x_sb[:, k, bass.ts(j, 128)], ident)
            if jp == 11:
                # the last pooled pair gates H: split its copy
                nc.vector.tensor_copy(xt[:, 22:23, :], tp[:, 0:1])
                nc.scalar.copy(xt[:, 23:24, :], tp[:, 1:2])
            elif jp % 2 == 0:
                # alternate engines: neither alone sustains the 728ns
                # x-half cadence once per-op overheads are counted
                nc.vector.tensor_copy(xt[:, 2 * jp:2 * jp + 2, :], tp)
            else:
                nc.scalar.copy(xt[:, 2 * jp:2 * jp + 2, :], tp)
            if idx >= 2 and T_ORDER[idx - 2] < 12:
                pool_mm(T_ORDER[idx - 2])
        pool_mm(10)
        pool_mm(11)

        # ---- H, G, hq (biases folded into psum as K=1 matmuls) ---------------
        # H lands in two psum tiles so the first half's copy-out (which
        # S1's k=0/1 matmuls wait on) does not WAR-block the second half's
        # matmuls on the same tile.
        h_sb = sb.tile([128, KT, 128], F16)
        for et in range(KT):
            # one psum tile (bank) per e-tile: et's copy-out must not
            # WAR-block or zero-region-clobber the next et's accumulation
            hps = psA.tile([128, 128], F32, tag="a")
            for ck in range(KT):
                nc.tensor.matmul(hps, mks[:, ck, bass.ts(et, 128)],
                                 xp[:, ck, :], start=(ck == 0), stop=False)
            nc.tensor.matmul(hps, vkr[:, bass.ts(et, 128)], onesr,
                             start=False, stop=True)
            if et % 2 == 0:
                nc.scalar.copy(h_sb[:, et, :], hps)
            else:
                nc.vector.tensor_copy(h_sb[:, et, :], hps)
        g_sb = sb.tile([128, KT, 128], F16)
        hq = sb.tile([128, 1], F32)

        def emit_g():
            # deferred past s1_stage(0): G waits on the late mqs DMA and
            # must not gate S1 through PE program order
            gps = psA.tile([128, KT, 128], F32, tag="a")
            for et in range(KT):
                for ck in range(KT):
                    nc.tensor.matmul(gps[:, et, :],
                                     mqs[:, ck, bass.ts(et, 128)],
                                     xp[:, ck, :],
                                     start=(ck == 0), stop=False)
                nc.tensor.matmul(gps[:, et, :],
                                 vqr[:, bass.ts(et, 128)], onesr,
                                 start=False, stop=True)
            nc.vector.tensor_copy(g_sb, gps)

        def emit_hq():
            hqps = psA.tile([128, 2], F32, tag="a")
            for ck in range(KT):
                nc.tensor.matmul(hqps, xp[:, ck, :], vq2[:, ck, :],
                                 start=(ck == 0), stop=False)
            nc.tensor.matmul(hqps, onesr, hb2, start=False, stop=True)
            nc.vector.tensor_copy(hq, hqps[:, 0:1])

        # ---- pass 1: S1 -> E1 -> (xbar) E1T -> M1, software-pipelined --------
        # M1 for chunk a is issued after S1 for chunk a+3, so the PE never
        # stalls on the exp + xbar-transpose round trip.
        rs1 = sb.tile([128, NCH], F32)
        m1tps = []
        for ek in range(KT):
            m1b = psB.tile([128, 128], F32, tag="b")
            m1tps.append(m1b)
        e1ts = []
        e2s = []
        eng_ns = [0.0, 0.0]          # accumulated Act / DVE pass-2 time
        SCALE_COST = (612.0, 658.0)  # Act identity-scale / DVE tsp mul

        def s1_stage(a):
            s1 = psA.tile([128, 512], F32, tag="a")
            for k in range(KT):
                nc.tensor.matmul(s1, h_sb[:, k, :], x_sb[:, k, bass.ts(a, 512)],
                                 start=(k == 0), stop=(k == KT - 1))
            e1 = e1p.tile([128, 512], BF16, tag="e1")
            nc.scalar.activation(out=e1, in_=s1,
                                 func=mybir.ActivationFunctionType.Exp,
                                 bias=sh1, scale=1.0,
                                 accum_out=rs1[:, a:a + 1])
            e1t = e1tp.tile([128, 4, 128], BF16, tag="e1t")
            nc.sync.dma_start_transpose(e1t, e1)
            e1ts.append(e1t)

        def s2_stage(a):
            s2 = psA.tile([128, 512], F32, tag="a")
            for k in range(KT):
                nc.tensor.matmul(s2, g_sb[:, k, :], x_sb[:, k, bass.ts(a, 512)],
                                 start=(k == 0), stop=(k == KT - 1))
            e2 = e2p.tile([128, 512], BF16, tag="e2")
            nc.scalar.activation(out=e2, in_=s2,
                                 func=mybir.ActivationFunctionType.Exp,
                                 bias=hq, scale=1.0)
            eng_ns[0] += 611.0
            e2s.append(e2)

        def m1_stage(a, split_last=False):
            e1t = e1ts[a]
            if not split_last:
                for u in range(4):
                    j = 4 * a + u
                    for ek in range(KT):
                        nc.tensor.matmul(m1tps[ek], xt[:, j, bass.ts(ek, 128)],
                                         e1t[:, u, :],
                                         start=(j == 0), stop=False)
            else:
                # bank-major order so each m1t bank closes (and can be
                # copied out) while PE still works on the next bank
                for ek in range(KT):
                    for u in range(4):
                        j = 4 * a + u
                        nc.tensor.matmul(m1tps[ek], xt[:, j, bass.ts(ek, 128)],
                                         e1t[:, u, :],
                                         start=False, stop=(u == 3))
                    if ek % 2 == 0:
                        nc.scalar.copy(m1t[:, ek, :], m1tps[ek])
                    else:
                        nc.vector.tensor_copy(m1t[:, ek, :], m1tps[ek])

        m1t = sb.tile([128, KT, 128], BF16)
        s1_stage(0)
        emit_g()
        s1_stage(1)
        emit_hq()
        for a in range(2, NCH):
            s1_stage(a)
        s2_stage(0)
        s2_stage(1)
        for a in range(NCH - 1):
            m1_stage(a)
            if a == 1:
                rsum1 = sb.tile([128, 1], F32)
                nc.vector.reduce_sum(out=rsum1, in_=rs1,
                                     axis=mybir.AxisListType.X)
                inv1 = sb.tile([128, 1], F32)
                nc.vector.reciprocal(inv1, rsum1)
        m1_stage(NCH - 1, split_last=True)

        # ---- c = (M1 @ Wv^T)/rowsum1 + bv ------------------------------------
        cps = psA.tile([128, D], F32, tag="a")
        for i in range(KT):
            nc.tensor.matmul(cps, m1t[:, i, :], wvt[:, i, :],
                             start=(i == 0), stop=(i == KT - 1))
        cbv = sb.tile([128, D], BF16)
        nc.vector.scalar_tensor_tensor(out=cbv, in0=cps, scalar=inv1, in1=bvb,
                                       op0=mybir.AluOpType.mult,
                                       op1=mybir.AluOpType.add)

        # ---- pass 2: S2 -> E2 -> out = (E2^T cbv) * inv2, software-pipelined -
        or_ = OUT.rearrange("(c j p) d -> c p j d", j=4, p=128)
        inv2s = []

        def rs2_stage(a):
            e2 = e2s[a]
            rsps = psB.tile([128, 4, 2], F32, tag="b")
            for u in range(4):
                nc.tensor.matmul(rsps[:, u, :], e2[:, bass.ts(u, 128)], ones2,
                                 start=True, stop=True)
            inv2 = iv2p.tile([128, 4], F32, tag="iv2")
            nc.vector.reciprocal(inv2, rsps[:, :, 0])
            eng_ns[1] += 129.0
            inv2s.append(inv2)

        def out_stage(a):
            e2 = e2s[a]
            inv2 = inv2s[a]
            last = a == NCH - 1
            o_sb = outp.tile([128, 4, D], F16, tag="o")
            for u in range(4):
                # odd u-tiles borrow psB so neither psum ring becomes the
                # per-chunk bottleneck (psB otherwise only holds tiny rsps)
                pool, ptag = (psA, "a") if u % 2 == 0 else (psB, "b")
                ops = pool.tile([128, D], F32, tag=ptag)
                nc.tensor.matmul(ops, e2[:, bass.ts(u, 128)], cbv,
                                 start=True, stop=True)
                if last:
                    eng = 1 if u % 2 == 0 else 0
                else:
                    eng = min((0, 1),
                              key=lambda i: eng_ns[i] + SCALE_COST[i])
                eng_ns[eng] += SCALE_COST[eng]
                if eng == 0:
                    nc.scalar.activation(
                        out=o_sb[:, u, :], in_=ops,
                        func=mybir.ActivationFunctionType.Identity,
                        bias=0.0, scale=inv2[:, u:u + 1])
                else:
                    nc.vector.tensor_scalar_mul(o_sb[:, u, :], ops,
                                                inv2[:, u:u + 1])
                if u == 1:
                    nc.sync.dma_start(out=or_[a][:, 0:2, :],
                                      in_=o_sb[:, 0:2, :])
            nc.sync.dma_start(out=or_[a][:, 2:4, :], in_=o_sb[:, 2:4, :])

        rs2_stage(0)
        for a in range(NCH):
            if a + 2 < NCH:
                s2_stage(a + 2)
            if a + 1 < NCH:
                rs2_stage(a + 1)
            out_stage(a)

    nc.compile()
    return nc


def _host_inputs(x, Wq, bq, Wk, bk, Wv, bv):
    del bk  # stage-1 softmax is invariant to the k-projection bias
    Wq = np.asarray(Wq, dtype=np.float32)
    Wk = np.asarray(Wk, dtype=np.float32)
    Wv = np.asarray(Wv, dtype=np.float32)
    bq = np.asarray(bq, dtype=np.float32)
    bv = np.asarray(bv, dtype=np.float32)
    bf16 = ml_dtypes.bfloat16
    inds = np.zeros((128, 4), dtype=np.float32)
    inds[np.arange(128), np.arange(128) // SEG] = 1.0
    ones2 = np.zeros((128, 2), dtype=np.float32)
    ones2[:, 0] = 1.0
    common = {
        "mks": ((Wq.T @ Wk) / SEG).astype(np.float16),
        "mqs": ((Wq.T @ Wq) / SEG).astype(np.float16),
        "wvt": np.ascontiguousarray(Wv.T).astype(bf16),
        "ident": np.eye(128, dtype=np.float16),
        "inds": inds.astype(bf16),
        "ones2": ones2.astype(bf16),
        "vkr": (Wk.T @ bq).astype(np.float16),
        "vqr": (Wq.T @ bq).astype(np.float16),
        "vq2": np.stack([(Wq.T @ bq) / SEG, np.zeros(D, np.float32)],
                        axis=1).astype(np.float16),
        "hb2": np.array([float(bq @ bq) - SHIFT2, 0.0], dtype=np.float16),
        "onesr": np.ones(128, dtype=np.float16),
        "bvb": np.tile(bv[None, :], (128, 1)).astype(bf16),
    }
    maps = []
    for b in range(B):
        m = dict(common)
        m["x"] = np.ascontiguousarray(x[b]).astype(np.float16)
        maps.append(m)
    return maps


def kernel(x, Wq, bq, Wk, bk, Wv, bv):
    x = np.asarray(x, dtype=np.float32)
    if "nc" not in _CACHE:
        _CACHE["nc"] = build()
    nc = _CACHE["nc"]
    in_maps = _host_inputs(x, Wq, bq, Wk, bk, Wv, bv)
    res = run_bass_kernel_spmd(nc, in_maps, core_ids=list(range(B)))
    out = np.empty((B, D, L), dtype=np.float32)
    for b in range(B):
        out[b] = np.asarray(res.results[b]["out"]).astype(np.float32).T
    return out


# revision 52
# speedup vs baseline: 1.0167x; 1.0052x over previous
"""AgentSelfAttention1d Trainium2 kernel (v3).

Per batch b (one NeuronCore each):
    xt = x[b].T                       # [L=4096, D=512]
    q/k/v = xt @ W{q,k,v}.T + b       # [L, D]
    a  = AdaptiveAvgPool(q) -> [P=128, D]
    c  = softmax(a @ k.T, -1) @ v     # [P, D]
    r  = softmax(q @ a.T, -1) @ c     # [L, D]
    out[b] = r.T                      # [D, L]

Restructuring (all projections folded into host-precomputed weight
products; everything channel-first on chip):
    xp[c,p]   = seg-sum of x over 32-wide windows      (tiny PE matmuls
                against a one-hot segment indicator, from the x.T tiles)
    H[e,p]    = MKs[c,e]^T-contract xp + vk[e],  MKs = (Wq^T Wk)/32,
                vk = Wk^T bq          (S1[p,l] = sum_e H[e,p] x[e,l])
    G[e,p]    = MQs-contract xp + vq[e],         MQs = (Wq^T Wq)/32
    hq[p]     = (xp^T (Wq^T bq))/32 + |bq|^2     (S2[p,l] = G-part + hq)
    E1        = exp(S1 - 10)  bf16; rowsum via activation accumulator
    E2        = exp(S2 - 40)  bf16 (unnormalized; bf16 keeps the f32
                exponent range so all-underflow columns cannot occur)
    M1T[e,p]  = (E1 @ x.T)^T, accumulated directly transposed across four
                PSUM banks; E1 transposed by the DMA xbar engine
    cbv[p,d]  = (M1 @ Wv^T) / rowsum1 + bv
    out[l,d]  = (sum_p E2[p,l] cbv[p,d]) / colsum2[l]
    Output written [L, D] fp16; host transposes/upcasts to [D, L] f32.

v3 schedule: x DMAs are issued first (ident ahead of them on the sync
queue) so transposes start ~2.5us in; all 8 chunks are PE-transposed and
pooled uniformly during the x-load window (no DVE/xbar special case for
chunk 7, which previously sat on the critical path after the last x
chunk); pass 1 interleaves S1 and S2 per chunk (E1 and E2 exps spread
across the whole pass on the Act engine) with M1 trailing two chunks;
pass 2 is only the 32 output matmuls with colsum2 (rs2) one chunk ahead,
scales round-robined over Act/DVE/GpSimd, and the final chunk stored as
four small DMAs to cut the tail.
"""

import numpy as np
import ml_dtypes

import concourse.bass as bass
import concourse.mybir as mybir
import concourse.tile as tile
from concourse import bacc
from concourse.bass_utils import run_bass_kernel_spmd

F32 = mybir.dt.float32
F16 = mybir.dt.float16
BF16 = mybir.dt.bfloat16

B, D, L, P = 8, 512, 4096, 128
KT = D // 128      # 4 contraction tiles of 128
NCH = L // 512     # 8 l-chunks of 512
NLT = L // 128     # 32 l-tiles of 128
SEG = L // P       # 32: pool segment length
SHIFT1 = 10.0
SHIFT2 = 40.0

_CACHE = {}


def build():
    nc = bacc.Bacc(target_bir_lowering=False, trn_type="TRN2")
    X = nc.dram_tensor("x", [D, L], F16, kind="ExternalInput")
    MKS = nc.dram_tensor("mks", [D, D], F16, kind="ExternalInput")   # (Wq^T Wk)/32 [c,e]
    MQS = nc.dram_tensor("mqs", [D, D], F16, kind="ExternalInput")   # (Wq^T Wq)/32 [c,e]
    WVT = nc.dram_tensor("wvt", [D, D], BF16, kind="ExternalInput")  # Wv^T [e,d]
    IDN = nc.dram_tensor("ident", [128, 128], F16, kind="ExternalInput")
    INDS = nc.dram_tensor("inds", [128, 4], BF16, kind="ExternalInput")  # l -> l//32 one-hot
    ONES2 = nc.dram_tensor("ones2", [128, 2], BF16, kind="ExternalInput")
    VKR = nc.dram_tensor("vkr", [D], F16, kind="ExternalInput")      # Wk^T bq row
    VQR = nc.dram_tensor("vqr", [D], F16, kind="ExternalInput")      # Wq^T bq row
    VQ2 = nc.dram_tensor("vq2", [D, 2], F16, kind="ExternalInput")   # [(Wq^T bq)/32, 0]
    HB2 = nc.dram_tensor("hb2", [2], F16, kind="ExternalInput")      # [|bq|^2-S2, 0]
    ONESR = nc.dram_tensor("onesr", [128], F16, kind="ExternalInput")
    BVB = nc.dram_tensor("bvb", [128, D], BF16, kind="ExternalInput")     # bv bcast
    OUT = nc.dram_tensor("out", [L, D], F16, kind="ExternalOutput")

    from contextlib import ExitStack
    with nc.allow_low_precision("16-bit matmul operands"), \
         tile.TileContext(nc, pool_alloc_mode="queue") as tc, ExitStack() as stack:
        sb = stack.enter_context(tc.tile_pool(name="sb", bufs=1))
        e1p = stack.enter_context(tc.tile_pool(name="e1p", bufs=8))
        e1tp = stack.enter_context(tc.tile_pool(name="e1tp", bufs=8))
        e2p = stack.enter_context(tc.tile_pool(name="e2p", bufs=8))
        outp = stack.enter_context(tc.tile_pool(name="outp", bufs=8))
        iv2p = stack.enter_context(tc.tile_pool(name="iv2p", bufs=8))
        # PSUM (8 banks): load: xpps 1 + tp ring | prep: h/g/hq ring |
        # pass1: s1+s2 ring 4 (psA) + m1t 4 (psB) | pass2: out 4 + rs2
        psA = stack.enter_context(tc.tile_pool(name="psA", bufs=4, space="PSUM"))
        psB = stack.enter_context(tc.tile_pool(name="psB", bufs=4, space="PSUM"))

        # ---- ACT table warmup ------------------------------------------------
        warm = sb.tile([128, 1], F32)
        nc.vector.memset(warm, 0.0)
        nc.scalar.activation(out=warm, in_=warm,
                             func=mybir.ActivationFunctionType.Exp,
                             bias=warm, scale=1.0)

        # ---- input DMAs: x first on sync; ident leads the SWDGE queue --------
        ident = sb.tile([128, 128], F16)
        nc.gpsimd.dma_start(out=ident, in_=IDN[:, :])
        x_sb = sb.tile([128, KT, L], F16)
        xr = X.rearrange("(k p) l -> p k l", p=128)
        # chunks 6,7 pool via DVE reduces (short post-arrival chain, ~1.35us)
        # so their halves can arrive late; chunk 6's ride mid-stream where
        # DVE has slack, chunk 7's come last among x. pool_mm chunks (0-5)
        # arrive early — their transpose->copy->pool chain needs ~2.2us.
        # mks follows x; H fires off its sem with no further dependencies.
        H_ORDER = [0, 1, 2, 3, 4, 5, 12, 6, 7, 8, 13, 9, 10, 11, 14, 15]
        for hch in H_ORDER:
            nc.sync.dma_start(out=x_sb[:, :, bass.ts(hch, 256)],
                              in_=xr[:, :, bass.ts(hch, 256)])
        # small tensors ride the SWDGE queue (prep overlaps the x stream)
        inds = sb.tile([128, 4], BF16)
        nc.gpsimd.dma_start(out=inds, in_=INDS[:, :])
        vkr = sb.tile([1, D], F16)
        nc.gpsimd.dma_start(out=vkr, in_=VKR.rearrange("(o d) -> o d", o=1))
        vqr = sb.tile([1, D], F16)
        nc.gpsimd.dma_start(out=vqr, in_=VQR.rearrange("(o d) -> o d", o=1))
        vq2 = sb.tile([128, KT, 2], F16)
        nc.gpsimd.dma_start(out=vq2, in_=VQ2.rearrange("(k p) t -> p k t", p=128))
        hb2 = sb.tile([1, 2], F16)
        nc.gpsimd.dma_start(out=hb2, in_=HB2.rearrange("(o d) -> o d", o=1))
        onesr = sb.tile([1, 128], F16)
        nc.gpsimd.dma_start(out=onesr, in_=ONESR.rearrange("(o d) -> o d", o=1))
        # weights after x on the sync queue; mks in two halves so H's first
        # contraction pair starts one transfer earlier
        mks = sb.tile([128, KT, D], F16)
        mksr = MKS.rearrange("(k p) e -> p k e", p=128)
        nc.sync.dma_start(out=mks[:, 0:2, :], in_=mksr[:, 0:2, :])
        nc.sync.dma_start(out=mks[:, 2:4, :], in_=mksr[:, 2:4, :])
        mqs = sb.tile([128, KT, D], F16)
        nc.sync.dma_start(out=mqs, in_=MQS.rearrange("(k p) e -> p k e", p=128))
        wvt = sb.tile([128, KT, D], BF16)
        nc.sync.dma_start(out=wvt, in_=WVT.rearrange("(k p) e -> p k e", p=128))
        ones2 = sb.tile([128, 2], BF16)
        nc.sync.dma_start(out=ones2, in_=ONES2[:, :])
        bvb = sb.tile([128, D], BF16)
        nc.sync.dma_start(out=bvb, in_=BVB[:, :])
        sh1 = sb.tile([128, 1], F32)
        nc.vector.memset(sh1, -SHIFT1)

        # ---- x.T tiles (PE transpose) + pooling (tiny PE matmuls) ------------
        # All 8 chunks uniformly; runs inside the x-load window where PE,
        # Act, DVE and GpSimd are otherwise idle.
        xt = sb.tile([128, NLT, D], BF16)
        xpps = psA.tile([128, KT, 128], F32, tag="a")
        xp = sb.tile([128, KT, 128], F16)

        def pool_mm(jp):
            for h in range(2):
                j = 2 * jp + h
                for t in range(KT):
                    nc.tensor.matmul(xpps[:, t, 4 * j:4 * j + 4],
                                     xt[:, j, bass.ts(t, 128)], inds,
                                     start=True, stop=True)
            if jp % 2 == 1:
                ch = (jp - 1) // 2
                if ch == 5:
                    # last pooled block gates H: split the copy
                    nc.scalar.copy(xp[:, :, 80:88], xpps[:, :, 80:88])
                    nc.vector.tensor_copy(xp[:, :, 88:96], xpps[:, :, 88:96])
                elif ch % 2 == 0:
                    nc.scalar.copy(xp[:, :, 16 * ch:16 * ch + 16],
                                   xpps[:, :, 16 * ch:16 * ch + 16])
                else:
                    nc.vector.tensor_copy(xp[:, :, 16 * ch:16 * ch + 16],
                                          xpps[:, :, 16 * ch:16 * ch + 16])

        # chunks 6 and 7 pool on DVE straight from x_sb (short post-arrival
        # chain); their halves arrive first so the four serial DVE reduces
        # hide inside the x stream. Their x.T tiles still come from the PE
        # transposes (only M1, much later, needs them).
        for hh in (12, 13, 14, 15):
            nc.vector.reduce_sum(
                out=xp[:, :, 8 * hh:8 * hh + 8],
                in_=x_sb[:, :, 256 * hh:256 * hh + 256].rearrange(
                    "p k (s t) -> p k s t", t=SEG),
                axis=mybir.AxisListType.X)
        NPAIR = NLT // 2   # 16 pairs, all chunks transposed
        T_ORDER = H_ORDER
        late_tps = []
        for idx, jp in enumerate(T_ORDER):
            tp = psB.tile([128, 2, 512], F16, tag="b")
            for h in range(2):
                j = 2 * jp + h
                for k in range(KT):
                    nc.tensor.transpose(tp[:, h, bass.ts(k, 128)],
                                        x_sb[:, k, bass.ts(j, 128)], ident)
            if jp >= 14:
                # chunk-7 pairs: copies deferred past the H-critical window
                # (only M1(7) needs them); their tp banks are recycled late,
                # which just makes the m1t accumulation start-wait on them
                late_tps.append((jp, tp))
            elif jp == 11:
                # the last pooled pair gates H: split its copy
                nc.vector.tensor_copy(xt[:, 22:23, :], tp[:, 0:1])
                nc.scalar.copy(xt[:, 23:24, :], tp[:, 1:2])
            elif jp % 2 == 0:
                # alternate engines: neither alone sustains the 728ns
                # x-half cadence once per-op overheads are counted
                nc.vector.tensor_copy(xt[:, 2 * jp:2 * jp + 2, :], tp)
            else:
                nc.scalar.copy(xt[:, 2 * jp:2 * jp + 2, :], tp)
            if idx >= 2 and T_ORDER[idx - 2] < 12:
                pool_mm(T_ORDER[idx - 2])

        # ---- H, G, hq (biases folded into psum as K=1 matmuls) ---------------
        # H lands in two psum tiles so the first half's copy-out (which
        # S1's k=0/1 matmuls wait on) does not WAR-block the second half's
        # matmuls on the same tile.
        h_sb = sb.tile([128, KT, 128], F16)
        for et in range(KT):
            # one psum tile (bank) per e-tile: et's copy-out must not
            # WAR-block or zero-region-clobber the next et's accumulation
            hps = psA.tile([128, 128], F32, tag="a")
            for ck in range(KT):
                nc.tensor.matmul(hps, mks[:, ck, bass.ts(et, 128)],
                                 xp[:, ck, :], start=(ck == 0), stop=False)
            nc.tensor.matmul(hps, vkr[:, bass.ts(et, 128)], onesr,
                             start=False, stop=True)
            if et % 2 == 0:
                nc.scalar.copy(h_sb[:, et, :], hps)
            else:
                nc.vector.tensor_copy(h_sb[:, et, :], hps)
        g_sb = sb.tile([128, KT, 128], F16)
        hq = sb.tile([128, 1], F32)

        def emit_g():
            # deferred past s1_stage(0): G waits on the late mqs DMA and
            # must not gate S1 through PE program order
            gps = psA.tile([128, KT, 128], F32, tag="a")
            for et in range(KT):
                for ck in range(KT):
                    nc.tensor.matmul(gps[:, et, :],
                                     mqs[:, ck, bass.ts(et, 128)],
                                     xp[:, ck, :],
                                     start=(ck == 0), stop=False)
                nc.tensor.matmul(gps[:, et, :],
                                 vqr[:, bass.ts(et, 128)], onesr,
                                 start=False, stop=True)
            nc.vector.tensor_copy(g_sb, gps)

        def emit_hq():
            hqps = psA.tile([128, 2], F32, tag="a")
            for ck in range(KT):
                nc.tensor.matmul(hqps, xp[:, ck, :], vq2[:, ck, :],
                                 start=(ck == 0), stop=False)
            nc.tensor.matmul(hqps, onesr, hb2, start=False, stop=True)
            nc.vector.tensor_copy(hq, hqps[:, 0:1])

        # ---- pass 1: S1 -> E1 -> (xbar) E1T -> M1, software-pipelined --------
        # M1 for chunk a is issued after S1 for chunk a+3, so the PE never
        # stalls on the exp + xbar-transpose round trip.
        rs1 = sb.tile([128, NCH], F32)
        m1tps = []
        for ek in range(KT):
            m1b = psB.tile([128, 128], F32, tag="b")
            m1tps.append(m1b)
        e1ts = []
        e2s = []
        eng_ns = [0.0, 0.0]          # accumulated Act / DVE pass-2 time
        SCALE_COST = (612.0, 658.0)  # Act identity-scale / DVE tsp mul

        def s1_stage(a):
            s1 = psA.tile([128, 512], F32, tag="a")
            for k in range(KT):
                nc.tensor.matmul(s1, h_sb[:, k, :], x_sb[:, k, bass.ts(a, 512)],
                                 start=(k == 0), stop=(k == KT - 1))
            e1 = e1p.tile([128, 512], BF16, tag="e1")
            nc.scalar.activation(out=e1, in_=s1,
                                 func=mybir.ActivationFunctionType.Exp,
                                 bias=sh1, scale=1.0,
                                 accum_out=rs1[:, a:a + 1])
            e1t = e1tp.tile([128, 4, 128], BF16, tag="e1t")
            nc.sync.dma_start_transpose(e1t, e1)
            e1ts.append(e1t)

        def s2_stage(a):
            s2 = psA.tile([128, 512], F32, tag="a")
            for k in range(KT):
                nc.tensor.matmul(s2, g_sb[:, k, :], x_sb[:, k, bass.ts(a, 512)],
                                 start=(k == 0), stop=(k == KT - 1))
            e2 = e2p.tile([128, 512], BF16, tag="e2")
            nc.scalar.activation(out=e2, in_=s2,
                                 func=mybir.ActivationFunctionType.Exp,
                                 bias=hq, scale=1.0)
            eng_ns[0] += 611.0
            e2s.append(e2)

        def m1_stage(a, split_last=False):
            e1t = e1ts[a]
            if not split_last:
                for u in range(4):
                    j = 4 * a + u
                    for ek in range(KT):
                        nc.tensor.matmul(m1tps[ek], xt[:, j, bass.ts(ek, 128)],
                                         e1t[:, u, :],
                                         start=(j == 0), stop=False)
            else:
                # bank-major order so each m1t bank closes (and can be
                # copied out) while PE still works on the next bank
                for ek in range(KT):
                    for u in range(4):
                        j = 4 * a + u
                        nc.tensor.matmul(m1tps[ek], xt[:, j, bass.ts(ek, 128)],
                                         e1t[:, u, :],
                                         start=False, stop=(u == 3))
                    if ek % 2 == 0:
                        nc.scalar.copy(m1t[:, ek, :], m1tps[ek])
                    else:
                        nc.vector.tensor_copy(m1t[:, ek, :], m1tps[ek])

        m1t = sb.tile([128, KT, 128], BF16)
        s1_stage(0)
        emit_g()
        s1_stage(1)
        emit_hq()
        for jp, tp in late_tps:
            if jp % 2 == 0:
                nc.vector.tensor_copy(xt[:, 2 * jp:2 * jp + 2, :], tp)
            else:
                nc.scalar.copy(xt[:, 2 * jp:2 * jp + 2, :], tp)
        for a in range(2, NCH):
            s1_stage(a)
        s2_stage(0)
        s2_stage(1)
        for a in range(NCH - 1):
            m1_stage(a)
            if a == 1:
                rsum1 = sb.tile([128, 1], F32)
                nc.vector.reduce_sum(out=rsum1, in_=rs1,
                                     axis=mybir.AxisListType.X)
                inv1 = sb.tile([128, 1], F32)
                nc.vector.reciprocal(inv1, rsum1)
        m1_stage(NCH - 1, split_last=True)

        # ---- c = (M1 @ Wv^T)/rowsum1 + bv ------------------------------------
        cps = psA.tile([128, D], F32, tag="a")
        for i in range(KT):
            nc.tensor.matmul(cps, m1t[:, i, :], wvt[:, i, :],
                             start=(i == 0), stop=(i == KT - 1))
        cbv = sb.tile([128, D], BF16)
        nc.vector.scalar_tensor_tensor(out=cbv, in0=cps, scalar=inv1, in1=bvb,
                                       op0=mybir.AluOpType.mult,
                                       op1=mybir.AluOpType.add)

        # ---- pass 2: S2 -> E2 -> out = (E2^T cbv) * inv2, software-pipelined -
        or_ = OUT.rearrange("(c j p) d -> c p j d", j=4, p=128)
        inv2s = []

        def rs2_stage(a):
            e2 = e2s[a]
            rsps = psB.tile([128, 4, 2], F32, tag="b")
            for u in range(4):
                nc.tensor.matmul(rsps[:, u, :], e2[:, bass.ts(u, 128)], ones2,
                                 start=True, stop=True)
            inv2 = iv2p.tile([128, 4], F32, tag="iv2")
            nc.vector.reciprocal(inv2, rsps[:, :, 0])
            eng_ns[1] += 129.0
            inv2s.append(inv2)

        def out_stage(a):
            e2 = e2s[a]
            inv2 = inv2s[a]
            last = a == NCH - 1
            o_sb = outp.tile([128, 4, D], F16, tag="o")
            for u in range(4):
                # odd u-tiles borrow psB so neither psum ring becomes the
                # per-chunk bottleneck (psB otherwise only holds tiny rsps)
                pool, ptag = (psA, "a") if u % 2 == 0 else (psB, "b")
                ops = pool.tile([128, D], F32, tag=ptag)
                nc.tensor.matmul(ops, e2[:, bass.ts(u, 128)], cbv,
                                 start=True, stop=True)
                if last:
                    eng = 1 if u % 2 == 0 else 0
                else:
                    eng = min((0, 1),
                              key=lambda i: eng_ns[i] + SCALE_COST[i])
                eng_ns[eng] += SCALE_COST[eng]
                if eng == 0:
                    nc.scalar.activation(
                        out=o_sb[:, u, :], in_=ops,
                        func=mybir.ActivationFunctionType.Identity,
                        bias=0.0, scale=inv2[:, u:u + 1])
                else:
                    nc.vector.tensor_scalar_mul(o_sb[:, u, :], ops,
                                                inv2[:, u:u + 1])
                if u == 1:
                    nc.sync.dma_start(out=or_[a][:, 0:2, :],
                                      in_=o_sb[:, 0:2, :])
            nc.sync.dma_start(out=or_[a][:, 2:4, :], in_=o_sb[:, 2:4, :])

        rs2_stage(0)
        for a in range(NCH):
            if a + 2 < NCH:
                s2_stage(a + 2)
            if a + 1 < NCH:
                rs2_stage(a + 1)
            out_stage(a)

    nc.compile()
    return nc


def _host_inputs(x, Wq, bq, Wk, bk, Wv, bv):
    del bk  # stage-1 softmax is invariant to the k-projection bias
    Wq = np.asarray(Wq, dtype=np.float32)
    Wk = np.asarray(Wk, dtype=np.float32)
    Wv = np.asarray(Wv, dtype=np.float32)
    bq = np.asarray(bq, dtype=np.float32)
    bv = np.asarray(bv, dtype=np.float32)
    bf16 = ml_dtypes.bfloat16
    inds = np.zeros((128, 4), dtype=np.float32)
    inds[np.arange(128), np.arange(128) // SEG] = 1.0
    ones2 = np.zeros((128, 2), dtype=np.float32)
    ones2[:, 0] = 1.0
    common = {
        "mks": ((Wq.T @ Wk) / SEG).astype(np.float16),
        "mqs": ((Wq.T @ Wq) / SEG).astype(np.float16),
        "wvt": np.ascontiguousarray(Wv.T).astype(bf16),
        "ident": np.eye(128, dtype=np.float16),
        "inds": inds.astype(bf16),
        "ones2": ones2.astype(bf16),
        "vkr": (Wk.T @ bq).astype(np.float16),
        "vqr": (Wq.T @ bq).astype(np.float16),
        "vq2": np.stack([(Wq.T @ bq) / SEG, np.zeros(D, np.float32)],
                        axis=1).astype(np.float16),
        "hb2": np.array([float(bq @ bq) - SHIFT2, 0.0], dtype=np.float16),
        "onesr": np.ones(128, dtype=np.float16),
        "bvb": np.tile(bv[None, :], (128, 1)).astype(bf16),
    }
    maps = []
    for b in range(B):
        m = dict(common)
        m["x"] = np.ascontiguousarray(x[b]).astype(np.float16)
        maps.append(m)
    return maps


def kernel(x, Wq, bq, Wk, bk, Wv, bv):
    x = np.asarray(x, dtype=np.float32)
    if "nc" not in _CACHE:
        _CACHE["nc"] = build()
    nc = _CACHE["nc"]
    in_maps = _host_inputs(x, Wq, bq, Wk, bk, Wv, bv)
    res = run_bass_kernel_spmd(nc, in_maps, core_ids=list(range(B)))
    out = np.empty((B, D, L), dtype=np.float32)
    for b in range(B):
        out[b] = np.asarray(res.results[b]["out"]).astype(np.float32).T
    return out


# revision 58
# speedup vs baseline: 1.0433x; 1.0262x over previous
"""AgentSelfAttention1d Trainium2 kernel (v3).

Per batch b (one NeuronCore each):
    xt = x[b].T                       # [L=4096, D=512]
    q/k/v = xt @ W{q,k,v}.T + b       # [L, D]
    a  = AdaptiveAvgPool(q) -> [P=128, D]
    c  = softmax(a @ k.T, -1) @ v     # [P, D]
    r  = softmax(q @ a.T, -1) @ c     # [L, D]
    out[b] = r.T                      # [D, L]

Restructuring (all projections folded into host-precomputed weight
products; everything channel-first on chip):
    xp[c,p]   = seg-sum of x over 32-wide windows      (tiny PE matmuls
                against a one-hot segment indicator, from the x.T tiles)
    H[e,p]    = MKs[c,e]^T-contract xp + vk[e],  MKs = (Wq^T Wk)/32,
                vk = Wk^T bq          (S1[p,l] = sum_e H[e,p] x[e,l])
    G[e,p]    = MQs-contract xp + vq[e],         MQs = (Wq^T Wq)/32
    hq[p]     = (xp^T (Wq^T bq))/32 + |bq|^2     (S2[p,l] = G-part + hq)
    E1        = exp(S1 - 10)  bf16; rowsum via activation accumulator
    E2        = exp(S2 - 40)  bf16 (unnormalized; bf16 keeps the f32
                exponent range so all-underflow columns cannot occur)
    M1T[e,p]  = (E1 @ x.T)^T, accumulated directly transposed across four
                PSUM banks; E1 transposed by the DMA xbar engine
    cbv[p,d]  = (M1 @ Wv^T) / rowsum1 + bv
    out[l,d]  = (sum_p E2[p,l] cbv[p,d]) / colsum2[l]
    Output written [L, D] fp16; host transposes/upcasts to [D, L] f32.

v3 schedule: x DMAs are issued first (ident ahead of them on the sync
queue) so transposes start ~2.5us in; all 8 chunks are PE-transposed and
pooled uniformly during the x-load window (no DVE/xbar special case for
chunk 7, which previously sat on the critical path after the last x
chunk); pass 1 interleaves S1 and S2 per chunk (E1 and E2 exps spread
across the whole pass on the Act engine) with M1 trailing two chunks;
pass 2 is only the 32 output matmuls with colsum2 (rs2) one chunk ahead,
scales round-robined over Act/DVE/GpSimd, and the final chunk stored as
four small DMAs to cut the tail.
"""

import numpy as np
import ml_dtypes

import concourse.bass as bass
import concourse.mybir as mybir
import concourse.tile as tile
from concourse import bacc
from concourse.bass_utils import run_bass_kernel_spmd

F32 = mybir.dt.float32
F16 = mybir.dt.float16
BF16 = mybir.dt.bfloat16

B, D, L, P = 8, 512, 4096, 128
KT = D // 128      # 4 contraction tiles of 128
NCH = L // 512     # 8 l-chunks of 512
NLT = L // 128     # 32 l-tiles of 128
SEG = L // P       # 32: pool segment length
SHIFT1 = 10.0
SHIFT2 = 40.0

_CACHE = {}


def build():
    nc = bacc.Bacc(target_bir_lowering=False, trn_type="TRN2")
    X = nc.dram_tensor("x", [D, L], F16, kind="ExternalInput")
    MKS = nc.dram_tensor("mks", [D, D], F16, kind="ExternalInput")   # (Wq^T Wk)/32 [c,e]
    MQS = nc.dram_tensor("mqs", [D, D], F16, kind="ExternalInput")   # (Wq^T Wq)/32 [c,e]
    WVT = nc.dram_tensor("wvt", [D, D], BF16, kind="ExternalInput")  # Wv^T [e,d]
    IDN = nc.dram_tensor("ident", [128, 128], F16, kind="ExternalInput")
    INDS = nc.dram_tensor("inds", [128, 4], BF16, kind="ExternalInput")  # l -> l//32 one-hot
    ONES2 = nc.dram_tensor("ones2", [128, 2], BF16, kind="ExternalInput")
    VKR = nc.dram_tensor("vkr", [D], F16, kind="ExternalInput")      # Wk^T bq row
    VQR = nc.dram_tensor("vqr", [D], F16, kind="ExternalInput")      # Wq^T bq row
    VQ2 = nc.dram_tensor("vq2", [D, 2], F16, kind="ExternalInput")   # [(Wq^T bq)/32, 0]
    HB2 = nc.dram_tensor("hb2", [2], F16, kind="ExternalInput")      # [|bq|^2-S2, 0]
    ONESR = nc.dram_tensor("onesr", [128], F16, kind="ExternalInput")
    BVB = nc.dram_tensor("bvb", [128, D], BF16, kind="ExternalInput")     # bv bcast
    OUT = nc.dram_tensor("out", [L, D], F16, kind="ExternalOutput")

    from contextlib import ExitStack
    with nc.allow_low_precision("16-bit matmul operands"), \
         tile.TileContext(nc, pool_alloc_mode="queue") as tc, ExitStack() as stack:
        sb = stack.enter_context(tc.tile_pool(name="sb", bufs=1))
        e1p = stack.enter_context(tc.tile_pool(name="e1p", bufs=8))
        e1tp = stack.enter_context(tc.tile_pool(name="e1tp", bufs=8))
        e2p = stack.enter_context(tc.tile_pool(name="e2p", bufs=8))
        outp = stack.enter_context(tc.tile_pool(name="outp", bufs=8))
        iv2p = stack.enter_context(tc.tile_pool(name="iv2p", bufs=8))
        # PSUM (8 banks): load: xpps 1 + tp ring | prep: h/g/hq ring |
        # pass1: s1+s2 ring 4 (psA) + m1t 4 (psB) | pass2: out 4 + rs2
        psA = stack.enter_context(tc.tile_pool(name="psA", bufs=4, space="PSUM"))
        psB = stack.enter_context(tc.tile_pool(name="psB", bufs=4, space="PSUM"))

        # ---- ACT table warmup ------------------------------------------------
        warm = sb.tile([128, 1], F32)
        nc.vector.memset(warm, 0.0)
        nc.scalar.activation(out=warm, in_=warm,
                             func=mybir.ActivationFunctionType.Exp,
                             bias=warm, scale=1.0)

        # ---- input DMAs: x first on sync; ident leads the SWDGE queue --------
        ident = sb.tile([128, 128], F16)
        nc.gpsimd.dma_start(out=ident, in_=IDN[:, :])
        x_sb = sb.tile([128, KT, L], F16)
        xr = X.rearrange("(k p) l -> p k l", p=128)
        # chunks 5,6,7 pool via DVE reduces (1.13us each, serial on DVE):
        # spread their halves through the stream so only the final half's
        # reduce runs after the stream ends. pool_mm chunks (0-4) arrive
        # early — their transpose->copy->pool chain needs ~2.2us. mks
        # follows x; H fires off max(mks sem, last reduce).
        H_ORDER = [0, 1, 2, 3, 10, 4, 5, 11, 6, 7, 12, 8, 9, 13, 14, 15]
        for hch in H_ORDER:
            nc.sync.dma_start(out=x_sb[:, :, bass.ts(hch, 256)],
                              in_=xr[:, :, bass.ts(hch, 256)])
        # small tensors ride the SWDGE queue (prep overlaps the x stream)
        inds = sb.tile([128, 4], BF16)
        nc.gpsimd.dma_start(out=inds, in_=INDS[:, :])
        vkr = sb.tile([1, D], F16)
        nc.gpsimd.dma_start(out=vkr, in_=VKR.rearrange("(o d) -> o d", o=1))
        vqr = sb.tile([1, D], F16)
        nc.gpsimd.dma_start(out=vqr, in_=VQR.rearrange("(o d) -> o d", o=1))
        vq2 = sb.tile([128, KT, 2], F16)
        nc.gpsimd.dma_start(out=vq2, in_=VQ2.rearrange("(k p) t -> p k t", p=128))
        hb2 = sb.tile([1, 2], F16)
        nc.gpsimd.dma_start(out=hb2, in_=HB2.rearrange("(o d) -> o d", o=1))
        onesr = sb.tile([1, 128], F16)
        nc.gpsimd.dma_start(out=onesr, in_=ONESR.rearrange("(o d) -> o d", o=1))
        # weights after x on the sync queue; mks in two halves so H's first
        # contraction pair starts one transfer earlier
        mks = sb.tile([128, KT, D], F16)
        mksr = MKS.rearrange("(k p) e -> p k e", p=128)
        nc.sync.dma_start(out=mks[:, 0:2, :], in_=mksr[:, 0:2, :])
        nc.sync.dma_start(out=mks[:, 2:4, :], in_=mksr[:, 2:4, :])
        mqs = sb.tile([128, KT, D], F16)
        nc.sync.dma_start(out=mqs, in_=MQS.rearrange("(k p) e -> p k e", p=128))
        wvt = sb.tile([128, KT, D], BF16)
        nc.sync.dma_start(out=wvt, in_=WVT.rearrange("(k p) e -> p k e", p=128))
        ones2 = sb.tile([128, 2], BF16)
        nc.sync.dma_start(out=ones2, in_=ONES2[:, :])
        bvb = sb.tile([128, D], BF16)
        nc.sync.dma_start(out=bvb, in_=BVB[:, :])
        sh1 = sb.tile([128, 1], F32)
        nc.vector.memset(sh1, -SHIFT1)

        # ---- x.T tiles (PE transpose) + pooling (tiny PE matmuls) ------------
        # All 8 chunks uniformly; runs inside the x-load window where PE,
        # Act, DVE and GpSimd are otherwise idle.
        xt = sb.tile([128, NLT, D], BF16)
        xpps = psA.tile([128, KT, 128], F32, tag="a")
        xp = sb.tile([128, KT, 128], F16)

        def pool_mm(jp):
            for h in range(2):
                j = 2 * jp + h
                for t in range(KT):
                    nc.tensor.matmul(xpps[:, t, 4 * j:4 * j + 4],
                                     xt[:, j, bass.ts(t, 128)], inds,
                                     start=True, stop=True)
            if jp % 2 == 1:
                # xp block copies all ride Act: DVE is reserved for the
                # segment reduces late in the stream
                ch = (jp - 1) // 2
                nc.scalar.copy(xp[:, :, 16 * ch:16 * ch + 16],
                               xpps[:, :, 16 * ch:16 * ch + 16])

        # chunks 5-7 pool on DVE straight from x_sb (short post-arrival
        # chain); their halves are spread through the stream so only the
        # final half's reduce runs after the stream ends. The reduces are
        # emitted inline at their arrival slots so they don't head-of-line
        # block the DVE pair copies. Their x.T tiles still come from the PE
        # transposes (only M1, much later, needs them).
        def seg_reduce(hh):
            nc.vector.reduce_sum(
                out=xp[:, :, 8 * hh:8 * hh + 8],
                in_=x_sb[:, :, 256 * hh:256 * hh + 256].rearrange(
                    "p k (s t) -> p k s t", t=SEG),
                axis=mybir.AxisListType.X)

        NPAIR = NLT // 2   # 16 pairs, all chunks transposed
        T_ORDER = H_ORDER
        late_tps = []
        for idx, jp in enumerate(T_ORDER):
            tp = psB.tile([128, 2, 512], F16, tag="b")
            for h in range(2):
                j = 2 * jp + h
                for k in range(KT):
                    nc.tensor.transpose(tp[:, h, bass.ts(k, 128)],
                                        x_sb[:, k, bass.ts(j, 128)], ident)
            if jp >= 14:
                # chunk-7 pairs: copies deferred past the H-critical window
                # (only M1(7) needs them); their tp banks are recycled late,
                # which just makes the m1t accumulation start-wait on them
                late_tps.append((jp, tp))
            elif jp in (12, 13) or jp % 2 == 1:
                # Act takes the odd pairs plus the late chunk-6 pairs: DVE
                # must be idle at stream end for the last reduces
                nc.scalar.copy(xt[:, 2 * jp:2 * jp + 2, :], tp)
            else:
                nc.vector.tensor_copy(xt[:, 2 * jp:2 * jp + 2, :], tp)
            if jp >= 10:
                seg_reduce(jp)
            if idx >= 2 and T_ORDER[idx - 2] < 10:
                pool_mm(T_ORDER[idx - 2])

        # ---- H, G, hq (biases folded into psum as K=1 matmuls) ---------------
        # H lands in two psum tiles so the first half's copy-out (which
        # S1's k=0/1 matmuls wait on) does not WAR-block the second half's
        # matmuls on the same tile.
        h_sb = sb.tile([128, KT, 128], F16)
        for et in range(KT):
            # one psum tile (bank) per e-tile: et's copy-out must not
            # WAR-block or zero-region-clobber the next et's accumulation
            hps = psA.tile([128, 128], F32, tag="a")
            for ck in range(KT):
                nc.tensor.matmul(hps, mks[:, ck, bass.ts(et, 128)],
                                 xp[:, ck, :], start=(ck == 0), stop=False)
            nc.tensor.matmul(hps, vkr[:, bass.ts(et, 128)], onesr,
                             start=False, stop=True)
            if et % 2 == 0:
                nc.scalar.copy(h_sb[:, et, :], hps)
            else:
                nc.vector.tensor_copy(h_sb[:, et, :], hps)
        g_sb = sb.tile([128, KT, 128], F16)
        hq = sb.tile([128, 1], F32)

        def emit_g():
            # deferred past s1_stage(0): G waits on the late mqs DMA and
            # must not gate S1 through PE program order
            gps = psA.tile([128, KT, 128], F32, tag="a")
            for et in range(KT):
                for ck in range(KT):
                    nc.tensor.matmul(gps[:, et, :],
                                     mqs[:, ck, bass.ts(et, 128)],
                                     xp[:, ck, :],
                                     start=(ck == 0), stop=False)
                nc.tensor.matmul(gps[:, et, :],
                                 vqr[:, bass.ts(et, 128)], onesr,
                                 start=False, stop=True)
            nc.vector.tensor_copy(g_sb, gps)

        def emit_hq():
            hqps = psA.tile([128, 2], F32, tag="a")
            for ck in range(KT):
                nc.tensor.matmul(hqps, xp[:, ck, :], vq2[:, ck, :],
                                 start=(ck == 0), stop=False)
            nc.tensor.matmul(hqps, onesr, hb2, start=False, stop=True)
            nc.vector.tensor_copy(hq, hqps[:, 0:1])

        # ---- pass 1: S1 -> E1 -> (xbar) E1T -> M1, software-pipelined --------
        # M1 for chunk a is issued after S1 for chunk a+3, so the PE never
        # stalls on the exp + xbar-transpose round trip.
        rs1 = sb.tile([128, NCH], F32)
        m1tps = []
        for ek in range(KT):
            m1b = psB.tile([128, 128], F32, tag="b")
            m1tps.append(m1b)
        e1ts = []
        e2s = []
        eng_ns = [0.0, 0.0]          # accumulated Act / DVE pass-2 time
        SCALE_COST = (612.0, 658.0)  # Act identity-scale / DVE tsp mul

        def s1_stage(a):
            s1 = psA.tile([128, 512], F32, tag="a")
            for k in range(KT):
                nc.tensor.matmul(s1, h_sb[:, k, :], x_sb[:, k, bass.ts(a, 512)],
                                 start=(k == 0), stop=(k == KT - 1))
            e1 = e1p.tile([128, 512], BF16, tag="e1")
            nc.scalar.activation(out=e1, in_=s1,
                                 func=mybir.ActivationFunctionType.Exp,
                                 bias=sh1, scale=1.0,
                                 accum_out=rs1[:, a:a + 1])
            e1t = e1tp.tile([128, 4, 128], BF16, tag="e1t")
            nc.sync.dma_start_transpose(e1t, e1)
            e1ts.append(e1t)

        def s2_stage(a):
            s2 = psA.tile([128, 512], F32, tag="a")
            for k in range(KT):
                nc.tensor.matmul(s2, g_sb[:, k, :], x_sb[:, k, bass.ts(a, 512)],
                                 start=(k == 0), stop=(k == KT - 1))
            e2 = e2p.tile([128, 512], BF16, tag="e2")
            nc.scalar.activation(out=e2, in_=s2,
                                 func=mybir.ActivationFunctionType.Exp,
                                 bias=hq, scale=1.0)
            eng_ns[0] += 611.0
            e2s.append(e2)

        def m1_stage(a, split_last=False):
            e1t = e1ts[a]
            if not split_last:
                for u in range(4):
                    j = 4 * a + u
                    for ek in range(KT):
                        nc.tensor.matmul(m1tps[ek], xt[:, j, bass.ts(ek, 128)],
                                         e1t[:, u, :],
                                         start=(j == 0), stop=False)
            else:
                # bank-major order so each m1t bank closes (and can be
                # copied out) while PE still works on the next bank
                for ek in range(KT):
                    for u in range(4):
                        j = 4 * a + u
                        nc.tensor.matmul(m1tps[ek], xt[:, j, bass.ts(ek, 128)],
                                         e1t[:, u, :],
                                         start=False, stop=(u == 3))
                    if ek % 2 == 0:
                        nc.scalar.copy(m1t[:, ek, :], m1tps[ek])
                    else:
                        nc.vector.tensor_copy(m1t[:, ek, :], m1tps[ek])

        m1t = sb.tile([128, KT, 128], BF16)
        s1_stage(0)
        emit_g()
        s1_stage(1)
        emit_hq()
        for jp, tp in late_tps:
            if jp % 2 == 0:
                nc.vector.tensor_copy(xt[:, 2 * jp:2 * jp + 2, :], tp)
            else:
                nc.scalar.copy(xt[:, 2 * jp:2 * jp + 2, :], tp)
        for a in range(2, NCH):
            s1_stage(a)
        s2_stage(0)
        s2_stage(1)
        for a in range(NCH - 1):
            m1_stage(a)
            if a == 1:
                rsum1 = sb.tile([128, 1], F32)
                nc.vector.reduce_sum(out=rsum1, in_=rs1,
                                     axis=mybir.AxisListType.X)
                inv1 = sb.tile([128, 1], F32)
                nc.vector.reciprocal(inv1, rsum1)
        m1_stage(NCH - 1, split_last=True)

        # ---- c = (M1 @ Wv^T)/rowsum1 + bv ------------------------------------
        cps = psA.tile([128, D], F32, tag="a")
        for i in range(KT):
            nc.tensor.matmul(cps, m1t[:, i, :], wvt[:, i, :],
                             start=(i == 0), stop=(i == KT - 1))
        cbv = sb.tile([128, D], BF16)
        nc.vector.scalar_tensor_tensor(out=cbv, in0=cps, scalar=inv1, in1=bvb,
                                       op0=mybir.AluOpType.mult,
                                       op1=mybir.AluOpType.add)

        # ---- pass 2: S2 -> E2 -> out = (E2^T cbv) * inv2, software-pipelined -
        or_ = OUT.rearrange("(c j p) d -> c p j d", j=4, p=128)
        inv2s = []

        def rs2_stage(a):
            e2 = e2s[a]
            rsps = psB.tile([128, 4, 2], F32, tag="b")
            for u in range(4):
                nc.tensor.matmul(rsps[:, u, :], e2[:, bass.ts(u, 128)], ones2,
                                 start=True, stop=True)
            inv2 = iv2p.tile([128, 4], F32, tag="iv2")
            nc.vector.reciprocal(inv2, rsps[:, :, 0])
            eng_ns[1] += 129.0
            inv2s.append(inv2)

        def out_stage(a):
            e2 = e2s[a]
            inv2 = inv2s[a]
            last = a == NCH - 1
            o_sb = outp.tile([128, 4, D], F16, tag="o")
            for u in range(4):
                # odd u-tiles borrow psB so neither psum ring becomes the
                # per-chunk bottleneck (psB otherwise only holds tiny rsps)
                pool, ptag = (psA, "a") if u % 2 == 0 else (psB, "b")
                ops = pool.tile([128, D], F32, tag=ptag)
                nc.tensor.matmul(ops, e2[:, bass.ts(u, 128)], cbv,
                                 start=True, stop=True)
                if last:
                    eng = 1 if u % 2 == 0 else 0
                else:
                    eng = min((0, 1),
                              key=lambda i: eng_ns[i] + SCALE_COST[i])
                eng_ns[eng] += SCALE_COST[eng]
                if eng == 0:
                    nc.scalar.activation(
                        out=o_sb[:, u, :], in_=ops,
                        func=mybir.ActivationFunctionType.Identity,
                        bias=0.0, scale=inv2[:, u:u + 1])
                else:
                    nc.vector.tensor_scalar_mul(o_sb[:, u, :], ops,
                                                inv2[:, u:u + 1])
                if u == 1:
                    nc.sync.dma_start(out=or_[a][:, 0:2, :],
                                      in_=o_sb[:, 0:2, :])
            nc.sync.dma_start(out=or_[a][:, 2:4, :], in_=o_sb[:, 2:4, :])

        rs2_stage(0)
        for a in range(NCH):
            if a + 2 < NCH:
                s2_stage(a + 2)
            if a + 1 < NCH:
                rs2_stage(a + 1)
            out_stage(a)

    nc.compile()
    return nc


def _host_inputs(x, Wq, bq, Wk, bk, Wv, bv):
    del bk  # stage-1 softmax is invariant to the k-projection bias
    Wq = np.asarray(Wq, dtype=np.float32)
    Wk = np.asarray(Wk, dtype=np.float32)
    Wv = np.asarray(Wv, dtype=np.float32)
    bq = np.asarray(bq, dtype=np.float32)
    bv = np.asarray(bv, dtype=np.float32)
    bf16 = ml_dtypes.bfloat16
    inds = np.zeros((128, 4), dtype=np.float32)
    inds[np.arange(128), np.arange(128) // SEG] = 1.0
    ones2 = np.zeros((128, 2), dtype=np.float32)
    ones2[:, 0] = 1.0
    common = {
        "mks": ((Wq.T @ Wk) / SEG).astype(np.float16),
        "mqs": ((Wq.T @ Wq) / SEG).astype(np.float16),
        "wvt": np.ascontiguousarray(Wv.T).astype(bf16),
        "ident": np.eye(128, dtype=np.float16),
        "inds": inds.astype(bf16),
        "ones2": ones2.astype(bf16),
        "vkr": (Wk.T @ bq).astype(np.float16),
        "vqr": (Wq.T @ bq).astype(np.float16),
        "vq2": np.stack([(Wq.T @ bq) / SEG, np.zeros(D, np.float32)],
                        axis=1).astype(np.float16),
        "hb2": np.array([float(bq @ bq) - SHIFT2, 0.0], dtype=np.float16),
        "onesr": np.ones(128, dtype=np.float16),
        "bvb": np.tile(bv[None, :], (128, 1)).astype(bf16),
    }
    maps = []
    for b in range(B):
        m = dict(common)
        m["x"] = np.ascontiguousarray(x[b]).astype(np.float16)
        maps.append(m)
    return maps


def kernel(x, Wq, bq, Wk, bk, Wv, bv):
    x = np.asarray(x, dtype=np.float32)
    if "nc" not in _CACHE:
        _CACHE["nc"] = build()
    nc = _CACHE["nc"]
    in_maps = _host_inputs(x, Wq, bq, Wk, bk, Wv, bv)
    res = run_bass_kernel_spmd(nc, in_maps, core_ids=list(range(B)))
    out = np.empty((B, D, L), dtype=np.float32)
    for b in range(B):
        out[b] = np.asarray(res.results[b]["out"]).astype(np.float32).T
    return out
